# revision 29
# baseline (speedup 1.0000x reference)
"""Multi-head attention (B=2, S=2048, H=1024, NH=16) on 8 TRN2 NeuronCores.

Sharding: core c -> (batch b = c//4, head-group hg = c%4). Each core computes
Q/K/V projections for its 4 heads (256 columns of Wq/Wk/Wv), attention for
those heads, and a partial output projection (its 256 rows of Wo). Host sums
the 4 partials per batch and adds bo.

Per-core device pipeline (all matmuls at 1 cycle/row via bf16):
  - x is pre-transposed + bf16-cast on the host, so h-major xT streams
    straight into the projections.
  - Q/K projections produce qT/kT d-major [256, 2048] (W stationary);
    V s-major [2048, 4, 65] bf16 (xT stationary) with a ones column appended.
  - scoresT[sk, sq] per head pair: lhsT = kT head slice (K=64); the two
    heads land in the two PSUM banks of one [128, 2, 512] tile.
  - softmax exp runs on BOTH ACT and DVE concurrently: ACT exps head 0's
    bank (table exp, scale=1/8 fused); DVE exps head 1's bank with a custom
    microcoded op (deg-3 Horner + 2 squarings ~ exp(x/8), max rel err 1.7%
    at the +-3.6-sigma tails, 0.4% in the bulk -- end-to-end rel err 0.006
    vs 2e-2 budget). This doubles exp bandwidth so the PE never starves in
    the attention phase; pre-computed full tiles during the projection
    phase (on both engines) cover the remaining shortfall.
  - AV in [sq, d] orientation: lhsT = et 128-col chunk, rhs = v+ones
    [128, 65] -> psum [sq 128, 65]; col 64 accumulates the softmax
    denominators for free.
  - Normalization: batched DVE reciprocal + TensorScalarPtr multiply per
    (head, sq-chunk) writes normalized attn [sq, d] bf16 out of PSUM.
  - attn -> attnT via DMA XBAR transpose on the SP queue.
  - Output projection all-bf16: attnT stationary, Wo rows moving; PSUM
    drained by ACT copies (no bias add on device -- host adds bo), y DMA'd
    out per 128-row chunk, chunks spread across the following batch.
PSUM budget (8 banks): 2 proj/outproj + 2x2 score double-buffer + 2 AV.
"""
import os
import sys

if os.path.isdir("/opt/trn_rl_repo"):
    sys.path.insert(0, "/opt/trn_rl_repo")

from contextlib import ExitStack

import numpy as np
import ml_dtypes

import concourse.bass as bass
import concourse.tile as tile
from concourse import bacc, mybir
from concourse.bass import ts
from concourse.bass_utils import run_bass_kernel_spmd

F32 = mybir.dt.float32
F32R = mybir.dt.float32r
BF16 = mybir.dt.bfloat16
EXP = mybir.ActivationFunctionType.Exp
COPY = mybir.ActivationFunctionType.Copy

S = 2048
H = 1024
D = 256          # per-core head-slice width (4 heads x 64)
HD = 64
N_CORES = 8
SB = 512         # s-block
NSB = S // SB    # 4
HT = H // 128    # 8 h-tiles
SKT = S // 128   # 16 sk-tiles
SCALE = 1.0 / 8.0  # 1/sqrt(HD)

# deg-3 Horner coefficients for the DVE exp op, raw-score basis:
# exp(u/8) ~ (((1 + u*(EC0 + u*(EC1 + u*EC2)))^2)^2, |u/8| <= 3.6
EC0 = 0.031503140926361084
EC1 = 0.0005148400668986142
EC2 = 4.80940570923849e-06

_CACHE = {}


def _register_exp_op():
    """Define + register the custom DVE op EXP_P3SQSQ_ANT (idempotent).
    Registration appends to dve_ops.OPS at runtime so the per-NEFF DVE
    table generation and row assignment pick it up without editing the
    repo."""
    if "exp_op" in _CACHE:
        return _CACHE["exp_op"]
    import concourse.dve_ops as dvo
    from concourse.dve_spec import Spec, Src0, C0, C1, C2, One, sq, lower, \
        _has_src1
    from concourse.dve_uop import DveOpSpec

    name = "EXP_P3SQSQ_ANT"
    for o in dvo.OPS:
        if o.name == name:
            _CACHE["exp_op"] = o
            return o
    body = sq(sq(One + Src0 * (C0 + Src0 * (C1 + Src0 * C2))))

    def ref(in0, in1, c0, c1, c2):
        p = (1.0 + in0 * (c0 + in0 * (c1 + in0 * c2))).astype(np.float32)
        p = (p * p).astype(np.float32)
        return (p * p).astype(np.float32)

    spec = Spec(body=body, reference=ref)
    row = dvo._CUSTOM_DVE_ROW_BASE + len(dvo.OPS)
    assert row < 0x20
    shas = {}
    for ver in ("v3", "v4"):
        try:
            uops = lower(spec, ver=ver)
            shas[ver] = DveOpSpec(name=name, opcode=row, uops=uops,
                                  rd1_en=_has_src1(spec)).sha(ver)
        except Exception:
            if ver == "v3":
                raise
    op = dvo.DveOp(name, spec, subdim=False, uops_sha=shas)
    dvo.OPS.append(op)
    dvo._SUB_OPCODE_FOR_NAME[name] = row
    dvo.CUSTOM_DVE_SPECS[name] = spec
    _CACHE["exp_op"] = op
    return op


def _build():
    exp_op = _register_exp_op()
    nc = bacc.Bacc("TRN2", target_bir_lowering=False, debug=False,
                   num_devices=N_CORES)

    xq = nc.dram_tensor("xqT", [H, S], BF16, kind="ExternalInput").ap()
    xk = nc.dram_tensor("xkT", [H, S], BF16, kind="ExternalInput").ap()
    xv = nc.dram_tensor("xvT", [H, S], BF16, kind="ExternalInput").ap()
    wq_d = nc.dram_tensor("wq", [H, D], BF16, kind="ExternalInput").ap()
    wk_d = nc.dram_tensor("wk", [H, D], BF16, kind="ExternalInput").ap()
    wv_d = nc.dram_tensor("wv", [H, D], BF16, kind="ExternalInput").ap()
    wo_d = nc.dram_tensor("wo", [D, H], BF16, kind="ExternalInput").ap()
    bq_d = nc.dram_tensor("bq2", [128, 2], F32, kind="ExternalInput").ap()
    bk_d = nc.dram_tensor("bk2", [128, 2], F32, kind="ExternalInput").ap()
    bv_d = nc.dram_tensor("bv1", [1, D], F32, kind="ExternalInput").ap()
    y = nc.dram_tensor("y", [S, H], BF16, kind="ExternalOutput").ap()

    def dve_exp(out_ap, in_ap):
        nc.vector._custom_dve(exp_op, out=out_ap, in0=in_ap,
                              s0=EC0, s1=EC1, imm2=EC2)

    with tile.TileContext(nc) as tc:
        with ExitStack() as ctx:
            const = ctx.enter_context(tc.tile_pool(name="const", bufs=1))
            pers = ctx.enter_context(tc.tile_pool(name="pers", bufs=1))
            xt_p = ctx.enter_context(tc.tile_pool(name="xt", bufs=2))
            small = ctx.enter_context(tc.tile_pool(name="small", bufs=4))
            pre_p = ctx.enter_context(tc.tile_pool(name="prep", bufs=94))
            eh_p = ctx.enter_context(tc.tile_pool(name="ehp", bufs=10))
            atn_p = ctx.enter_context(tc.tile_pool(name="atnp", bufs=6))
            fin_p = ctx.enter_context(tc.tile_pool(name="finp", bufs=4))

            # ---- constants ----
            # weights + biases go out on the ACT HWDGE queue so their
            # descriptor generation runs in parallel with the SP queue's
            # xt streams (two HWDGEs).
            wq = const.tile([128, HT, D], BF16)
            wq_r = wq_d.rearrange("(j p) d -> p j d", p=128)
            nc.scalar.dma_start(wq[:, :, 0:128], wq_r[:, :, 0:128])
            wk = const.tile([128, HT, D], BF16)
            wv = const.tile([128, HT, D], BF16)
            bq2 = const.tile([128, 2], F32)
            bk2 = const.tile([128, 2], F32)
            bv1 = const.tile([1, D], F32)
            ones_f = const.tile([1, 128], F32)
            nc.gpsimd.memset(ones_f[:], 1.0)
            ones = const.tile([1, 128], F32R)
            nc.vector.tensor_copy(ones[:], ones_f[:])
            bv1r = const.tile([1, D], F32R)
            warm = const.tile([1, 2], BF16)
            nc.scalar.activation(warm[:], ones_f[0:1, 0:2], EXP)
            # [128,128] bf16 identity for the tail's PE transposes
            id1 = const.tile([128, 128], BF16)
            nc.gpsimd.memset(id1[:], 1.0)
            ident = const.tile([128, 128], BF16)
            nc.gpsimd.affine_select(ident[:], id1[:], pattern=[[1, 128]],
                                    compare_op=mybir.AluOpType.is_equal,
                                    fill=0.0, base=0, channel_multiplier=-1)

            # ---- persistent activations ----
            qT = pers.tile([128, 2, S], BF16)   # [d_local, dh, s]
            kT = pers.tile([128, 2, S], BF16)
            vS = pers.tile([128, SKT, 4, HD + 1], BF16)  # [sk, skt, head, d|1]
            nc.gpsimd.memset(vS[:], 1.0)       # ones column (rest overwritten)
            attnT = pers.tile([128, 2, S], BF16)  # [d in pair, hp, sq]

            ps_pj = ctx.enter_context(
                tc.tile_pool(name="ps_pj", bufs=2, space="PSUM"))
            ps_qk = ctx.enter_context(
                tc.tile_pool(name="ps_qk", bufs=4, space="PSUM"))
            ps_av = ctx.enter_context(
                tc.tile_pool(name="ps_av", bufs=2, space="PSUM"))

            bvb = const.tile([128, D], F32)

            def load_xt(xd, sb, name):
                """DMA one s-block of pre-transposed x: [128h, HT, SB] bf16."""
                xt = xt_p.tile([128, HT, SB], BF16, tag="xt", name=name)
                nc.sync.dma_start(
                    xt[:], xd.rearrange("(j p) s -> p j s", p=128)[
                        :, :, ts(sb, SB)])
                return xt

            def proj_dmajor_unit(xt, w, bias2, dst, sb, dh, c0=0, c1=SB):
                # dst[:, dh, sb*SB+c0:+c1] = (x @ w + b).T (d-major)
                pp = ps_pj.tile([128, 512], F32, tag="pj", name="pp")
                for j in range(HT):
                    nc.tensor.matmul(pp[:, 0:c1 - c0], w[:, j, ts(dh, 128)],
                                     xt[:, j, c0:c1],
                                     start=(j == 0), stop=(j == HT - 1))
                nc.vector.tensor_scalar_add(
                    dst[:, dh, sb * SB + c0:sb * SB + c1], pp[:, 0:c1 - c0],
                    bias2[:, dh:dh + 1])

            def qk_score_h(hp, sqb, sk, hh):
                # one head's [128 sk, 512 sq] score tile: a single PSUM
                # bank, so the 4-deep ring gives two full tiles of exp
                # lookahead (the exp+semaphore round trip is ~1us while the
                # PE's per-tile work is ~0.65us).
                pqk = ps_qk.tile([128, 512], F32, tag="qk", name="pqk")
                r0 = HD * hh
                nc.tensor.matmul(
                    pqk[:],
                    kT[r0:r0 + HD, hp, ts(sk, 128)],
                    qT[r0:r0 + HD, hp, ts(sqb, SB)],
                    start=True, stop=True)
                return pqk

            def qk_exp_pre(hp, sqb, sk, eng="aa"):
                # pre-tile path (projection phase): per-head tiles, engine
                # per half given by `eng` (a=ACT, d=DVE).
                ets = []
                for hh in range(2):
                    pqk = qk_score_h(hp, sqb, sk, hh)
                    et = pre_p.tile([128, 512], BF16, tag="e", name="et")
                    if eng[hh] == "a":
                        nc.scalar.activation(et[:], pqk[:], EXP, scale=SCALE)
                    else:
                        dve_exp(et[:], pqk[:])
                    ets.append(et)
                return tuple(ets)

            def qk_exp_split(hp, sqb, sk):
                # inline path: head 0 -> ACT, head 1 -> DVE, concurrently.
                pqk0 = qk_score_h(hp, sqb, sk, 0)
                et0 = eh_p.tile([128, 512], BF16, tag="eh", name="et0")
                nc.scalar.activation(et0[:], pqk0[:], EXP, scale=SCALE)
                pqk1 = qk_score_h(hp, sqb, sk, 1)
                et1 = eh_p.tile([128, 512], BF16, tag="eh", name="et1")
                dve_exp(et1[:], pqk1[:])
                return (et0, et1)

            def av_accum(hp, sk, et, pav):
                # pav[hh][:, sqc, 0:65] += et[hh][:,chunk].T @ v+ones
                # start=True marks the whole 2KB PSUM bank pending-zero, so
                # only the bank's first group may use it; later groups'
                # first accumulate reads pending-zero bytes as zero.
                for hh in range(2):
                    for sqc in range(4):
                        nc.tensor.matmul(
                            pav[hh][:, sqc, 0:HD + 1],
                            et[hh][:, ts(sqc, 128)],
                            vS[:, sk, 2 * hp + hh, :],
                            start=(sk == 0 and sqc == 0),
                            stop=(sk == SKT - 1),
                            skip_group_check=True)

            def emit_outproj_mm(sqb, st):
                # po matmuls only; the ACT drain copies + y DMA are emitted
                # later (emit_outproj_drain) so they sit BEHIND the next exp
                # in the in-order ACT stream instead of blocking it.
                fin = fin_p.tile([128, H], BF16, tag="fin", name="fin")
                pos = []
                for eb in range(2):
                    po = ps_pj.tile([128, 512], F32, tag="pj", name="po")
                    nc.tensor.matmul(po[:],
                                     attnT[:, 0, ts(4 * sqb + st, 128)],
                                     wo[:, 0, ts(eb, 512)],
                                     start=True, stop=False,
                                     skip_group_check=True)
                    nc.tensor.matmul(po[:],
                                     attnT[:, 1, ts(4 * sqb + st, 128)],
                                     wo[:, 1, ts(eb, 512)],
                                     start=False, stop=True,
                                     skip_group_check=True)
                    pos.append(po)
                return (sqb, st, fin, pos)

            def emit_outproj_drain(w):
                # one PSUM->SBUF copy per engine so neither in-order queue
                # eats both
                sqb, st, fin, pos = w
                nc.scalar.activation(fin[:, ts(0, 512)], pos[0][:], COPY)
                nc.vector.tensor_copy(fin[:, ts(1, 512)], pos[1][:])
                nc.sync.dma_start(y[ts(4 * sqb + st, 128), :], fin[:])

            def attn_finish(hp, sqb, pav, fuse_outproj=False):
                # normalize out of PSUM (per-partition denominators in col 64)
                # and transpose [sq, d] -> attnT[d, sq] on the DMA XBAR (SP).
                # The per-chunk scale multiplies are split across DVE
                # (tensor_scalar) and ACT (Copy with per-partition scale) to
                # halve the batch-boundary normalize latency.
                atn = [atn_p.tile([128, 128], BF16, tag="atn", name="atn")
                       for _ in range(4)]
                recs = []
                for hh in range(2):
                    rec = small.tile([128, 4], F32, tag="rec", name="rec")
                    with nc.allow_low_precision(reason="softmax denom recip"):
                        nc.vector.reciprocal(rec[:], pav[hh][:, :, HD:HD + 1])
                    recs.append(rec)
                # tail fast-drain for the last batch: every chunk's head-0
                # outproj mm runs DURING the normalize+transpose window (its
                # attnT half landed a batch ago), on PSUM borrowed from the
                # now-idle qk/pj rings; chunk 3 takes the av banks once the
                # normalize reads drain. Chunk k then completes (head-1 mm +
                # drain + y DMA) as its transpose lands. Transposes alternate
                # SP/ACT queues so their configs don't serialize.
                tails = []
                if fuse_outproj:
                    for st in range(2):
                        fin = fin_p.tile([128, H], BF16, tag="fin",
                                         name="fin")
                        pos = []
                        for eb in range(2):
                            po = ps_qk.tile([128, 512], F32, tag="qk",
                                            name="pot")
                            nc.tensor.matmul(
                                po[:], attnT[:, 0, ts(4 * sqb + st, 128)],
                                wo[:, 0, ts(eb, 512)], start=True,
                                stop=False, skip_group_check=True)
                            pos.append(po)
                        tails.append((st, fin, pos))
                for sqc in range(4):
                    nc.vector.tensor_scalar_mul(
                        atn[sqc][:, ts(0, HD)],
                        pav[0][:, sqc, 0:HD],
                        recs[0][:, sqc:sqc + 1])
                    nc.scalar.activation(
                        atn[sqc][:, ts(1, HD)],
                        pav[1][:, sqc, 0:HD], COPY,
                        scale=recs[1][:, sqc:sqc + 1])
                    if not fuse_outproj:
                        nc.sync.dma_start_transpose(
                            attnT[:, hp, ts(4 * sqb + sqc, 128)],
                            atn[sqc][:])
                    else:
                        # tail: transpose on the PE (53ns + a short copy)
                        # instead of the ~2.3us DMA XBAR round trip; copies
                        # alternate DVE/ACT.
                        ptr = ps_pj.tile([128, 128], BF16, tag="pj",
                                         name="ptr")
                        nc.tensor.matmul(ptr[:], atn[sqc][:], ident[:],
                                         is_transpose=True,
                                         skip_group_check=True)
                        dst = attnT[:, hp, ts(4 * sqb + sqc, 128)]
                        if sqc % 2:
                            nc.scalar.activation(dst, ptr[:], COPY)
                        else:
                            nc.vector.tensor_copy(dst, ptr[:])
                if fuse_outproj:
                    # chunk 3's head-0 mms go to the av banks -- emitted
                    # after the muls above so the WAR on the freshly-read
                    # pav banks is tracked; chunk 2 reuses the pj ring after
                    # the transpose copies drain it.
                    for st in (3, 2):
                        fin = fin_p.tile([128, H], BF16, tag="fin",
                                         name="fin")
                        pos = []
                        for eb in range(2):
                            if st == 3:
                                pot = ps_av.tile([128, 4, 128], F32,
                                                 tag="av", name="pot")
                                po = pot.rearrange("p a b -> p (a b)")
                            else:
                                po = ps_pj.tile([128, 512], F32, tag="pj",
                                                name="pot")
                            nc.tensor.matmul(
                                po[:], attnT[:, 0, ts(4 * sqb + st, 128)],
                                wo[:, 0, ts(eb, 512)], start=True,
                                stop=False, skip_group_check=True)
                            pos.append(po)
                        tails.append((st, fin, pos))
                    tails.sort()
                    for st, fin, pos in tails:
                        for eb in range(2):
                            nc.tensor.matmul(
                                pos[eb][:],
                                attnT[:, 1, ts(4 * sqb + st, 128)],
                                wo[:, 1, ts(eb, 512)], start=False,
                                stop=True, skip_group_check=True)
                        nc.scalar.activation(fin[:, ts(0, 512)], pos[0][:],
                                             COPY)
                        nc.vector.tensor_copy(fin[:, ts(1, 512)], pos[1][:])
                        nc.sync.dma_start(y[ts(4 * sqb + st, 128), :],
                                          fin[:])

            # ---- streaming loads + projections, with scores+exp for ready
            # (sqb, hp, sk) tiles pre-emitted in consumption order so both
            # exp engines start chewing softmax work early. ----
            def spread(p):
                return sorted({int(round(i * SKT / p)) for i in range(p)})

            # pre-tiles: ACT-only (a DVE pre-exp would head-of-line-block
            # the projection epilogue adds on the in-order DVE queue and
            # stall the PE's ps_pj ring). Spread within each batch so the
            # inline ACT/DVE load stays even through the attention stream.
            # leading + trailing sks per batch: a pre-covered batch END lets
            # the engine queues drain before the normalize, so the boundary
            # WAR on the pav ring resolves fast; a pre-covered batch START
            # gives the PE immediate AV work after it. b00 (consumed inside
            # the projection phase) stays ACT-only; later pre tiles put one
            # half on DVE -- at most one 0.66us DVE op lands between
            # projection epilogue adds, within the pp ring's 2-unit slack.
            PRE_SPEC = [((0, 0), list(range(SKT)), "aa"),
                        ((0, 1), [0, 1, 13, 14, 15], "ad"),
                        ((1, 0), [0, 12, 13, 14, 15], "ad"),
                        ((1, 1), [0, 12, 13, 14, 15], "ad"),
                        ((2, 0), [0, 13, 14, 15], "ad"),
                        ((2, 1), [0, 13, 14, 15], "ad"),
                        ((3, 0), [0, 14, 15], "ad")]
            pre_order = []
            for (sqb, hp), sks, eng in PRE_SPEC:
                for sk in sks:
                    pre_order.append((sqb, hp, sk, eng))
            pre = {}
            st_pre = {"i": 0, "q": set(), "k": set()}

            def emit_pre(limit):
                # dh-granular readiness: head-pair hp only needs the dh=hp
                # halves of its qT/kT blocks.
                done = 0
                while st_pre["i"] < len(pre_order) and done < limit:
                    sqb, hp, sk, eng = pre_order[st_pre["i"]]
                    if (sqb, hp) not in st_pre["q"] or \
                            (sk // 4, hp) not in st_pre["k"]:
                        break
                    pre[(sqb, hp, sk)] = qk_exp_pre(hp, sqb, sk, eng)
                    st_pre["i"] += 1
                    done += 1

            def proj_qk(xd, w, bias2, dst, sb, which):
                xt = load_xt(xd, sb, "xt" + which)
                for dh in range(2):
                    proj_dmajor_unit(xt, w, bias2, dst, sb, dh)
                    st_pre[which].add((sb, dh))
                    emit_pre(3)

            # batch (0,0)'s AV interleaves into the V phase: its et tiles
            # are all precomputed, and vS[sk] is ready right after block
            # sk//4's epilogue -- so the first attention batch costs no
            # wall-clock of its own.
            pav00 = [ps_av.tile([128, 4, 128], F32, tag="av", name=f"pav{hh}")
                     for hh in range(2)]

            def proj_v(sb):
                # batch (0,0)'s AV trails the V epilogues by one si unit so
                # the in-order PE never waits on the DVE vS write latency.
                xtv = load_xt(xv, sb, "xtv")
                for si in range(4):
                    pv = ps_pj.tile([128, 512], F32, tag="pj", name="pv")
                    for j in range(HT):
                        nc.tensor.matmul(pv[:, 0:D],
                                         xtv[:, j, ts(si, 128)],
                                         wv[:, j, :],
                                         start=(j == 0), stop=(j == HT - 1))
                    nc.vector.tensor_add(
                        vS[:, 4 * sb + si, :, 0:HD],
                        pv[:, 0:D].rearrange("p (g d) -> p g d", g=4),
                        bvb[:].rearrange("p (g d) -> p g d", g=4))
                    emit_pre(3)
                    sk = 4 * sb + si
                    if sk > 0:
                        av_accum(0, sk - 1, pre.pop((0, 0, sk - 1)), pav00)

            # Q0 then all K (unlocks every sqb0 tile), then Q1-3 (unlocks
            # the rest), V last (first consumed only once attention starts).
            # xtq0 lands in two halves so the first projection matmul can
            # start on the first 256 columns while the rest streams in.
            xtq0 = xt_p.tile([128, HT, SB], BF16, tag="xt", name="xtq0")
            xq_r = xq.rearrange("(j p) s -> p j s", p=128)
            nc.sync.dma_start(xtq0[:, :, 0:256], xq_r[:, :, 0:256])
            nc.sync.dma_start(xtq0[:, :, 256:512], xq_r[:, :, 256:512])
            nc.scalar.dma_start(bq2[:], bq_d[:])
            wk_r = wk_d.rearrange("(j p) d -> p j d", p=128)
            nc.scalar.dma_start(wk[:, :, 0:128], wk_r[:, :, 0:128])
            # fast start: narrow first xk load + mini K projection puts the
            # first score+exp on the engines early.
            xtk0a = xt_p.tile([128, HT, 128], BF16, tag="xta", name="xtk0a", bufs=1)
            nc.sync.dma_start(wq[:, :, 128:256], wq_r[:, :, 128:256])
            nc.sync.dma_start(
                xtk0a[:], xk.rearrange("(j p) s -> p j s", p=128)[:, :, 0:128])
            nc.scalar.dma_start(bk2[:], bk_d[:])
            nc.scalar.dma_start(wk[:, :, 128:256], wk_r[:, :, 128:256])
            proj_dmajor_unit(xtq0, wq, bq2, qT, 0, 0, 0, 256)
            proj_dmajor_unit(xtq0, wq, bq2, qT, 0, 0, 256, 512)
            st_pre["q"].add((0, 0))
            pk0 = ps_pj.tile([128, 512], F32, tag="pj", name="pk0")
            for j in range(HT):
                nc.tensor.matmul(pk0[:, 0:128], wk[:, j, 0:128],
                                 xtk0a[:, j, :],
                                 start=(j == 0), stop=(j == HT - 1))
            nc.vector.tensor_scalar_add(kT[:, 0, 0:128], pk0[:, 0:128],
                                        bk2[:, 0:1])
            pre[(0, 0, 0)] = qk_exp_pre(0, 0, 0, "aa")
            proj_dmajor_unit(xtq0, wq, bq2, qT, 0, 1)
            st_pre["q"].add((0, 1))
            st_pre["i"] = 1
            xtk0 = load_xt(xk, 0, "xtk")
            proj_dmajor_unit(xtk0, wk, bk2, kT, 0, 0, 128, SB)
            st_pre["k"].add((0, 0))
            emit_pre(3)
            proj_dmajor_unit(xtk0, wk, bk2, kT, 0, 1)
            st_pre["k"].add((0, 1))
            emit_pre(3)
            for sb in range(1, NSB):
                proj_qk(xk, wk, bk2, kT, sb, "k")
                emit_pre(3)
            for sb in range(1, NSB):
                proj_qk(xq, wq, bq2, qT, sb, "q")
                emit_pre(3)
            nc.sync.dma_start(wv[:], wv_d.rearrange("(j p) d -> p j d", p=128))
            # v-bias broadcast, deferred here so its small DMAs stay off the
            # critical startup path (first needed by V0's epilogue)
            nc.sync.dma_start(bv1[:], bv_d[:])
            nc.vector.tensor_copy(bv1r[:], bv1[:])
            pbc = ps_pj.tile([128, 512], F32, tag="pj", name="pbc")
            nc.tensor.matmul(pbc[:, 0:D], ones[0:1, :], bv1r[:])
            nc.vector.tensor_copy(bvb[:], pbc[:, 0:D])
            for sb in range(NSB):
                proj_v(sb)
            av_accum(0, SKT - 1, pre.pop((0, 0, SKT - 1)), pav00)
            emit_pre(len(pre_order))

            # deferred: output-projection weights (first needed ~60us in)
            wo = const.tile([128, 2, H], BF16)
            nc.sync.dma_start(wo[:], wo_d.rearrange("(i p) e -> p i e", p=128))

            # ---- attention stream. Inline tiles split each head pair's
            # exp across ACT/DVE (separate PSUM banks), with 2-item score
            # lookahead across batch boundaries. outproj(sqb-1) chunks are
            # spread through the following batch (sk = 2,5,8,11) so the
            # ps_pj ring never backs up on the ACT drain. ----
            attn_finish(0, 0, pav00)
            batches = [(s, h) for s in range(NSB) for h in range(2)][1:]
            stream = [(s, h, k) for (s, h) in batches for k in range(SKT)]
            emitted = {}
            st_la = {"ep": 0}

            def pump(upto, inline_ahead=0):
                # emit score+exp for stream items <= upto, plus keep
                # `inline_ahead` INLINE (non-pre) tiles in flight beyond
                # the consumer. `emitted` holds exactly the un-consumed
                # inline tiles, so len(emitted) IS the in-flight count --
                # pre-covered items don't eat the lookahead window.
                while st_la["ep"] < len(stream) and \
                        (st_la["ep"] <= upto or len(emitted) < inline_ahead):
                    key = stream[st_la["ep"]]
                    if key not in pre:
                        emitted[key] = qk_exp_split(key[1], key[0], key[2])
                    st_la["ep"] += 1

            deferred = []
            pending_drain = []
            pav_cur = {}
            for ci, key in enumerate(stream):
                sqb, hp, sk = key
                if sk == 0:
                    pav_cur[(sqb, hp)] = [
                        ps_av.tile([128, 4, 128], F32, tag="av",
                                   name=f"pav{hh}") for hh in range(2)]
                # ensure the current tile's exp exists; the lookahead pump
                # at loop end runs AFTER any finish so normalize ops aren't
                # queued behind the next batch's exps.
                pump(ci)
                if pending_drain:
                    emit_outproj_drain(pending_drain.pop(0))
                et = pre.pop(key, None)
                if et is None:
                    et = emitted.pop(key)
                av_accum(hp, sk, et, pav_cur[(sqb, hp)])
                if deferred and sk in (4, 7, 10, 13):
                    dq, dst = deferred[0]
                    pending_drain.append(emit_outproj_mm(dq, dst))
                    if dst == 3:
                        deferred.pop(0)
                    else:
                        deferred[0] = (dq, dst + 1)
                if sk == SKT - 1:
                    last = (sqb == NSB - 1 and hp == 1)
                    attn_finish(hp, sqb, pav_cur.pop((sqb, hp)),
                                fuse_outproj=last)
                    if hp == 0 and sqb > 0:
                        deferred.append((sqb - 1, 0))
                pump(ci, inline_ahead=2)

    nc.compile()
    return nc


def _get_nc():
    if "nc" not in _CACHE:
        _CACHE["nc"] = _build()
    return _CACHE["nc"]


def _kernel_numpy(query, key, value, attention_mask,
                  Wq, bq, Wk, bk, Wv, bv, Wo, bo):
    """Exact fp32 numpy fallback (only used for inputs outside the spec:
    nonzero mask or unexpected shapes)."""
    B, S_, H_ = query.shape
    NH = 16
    HDl = H_ // NH
    q = query @ Wq + bq
    k = key @ Wk + bk
    v = value @ Wv + bv

    def split(x):
        return x.reshape(B, S_, NH, HDl).transpose(0, 2, 1, 3)

    q, k, v = split(q), split(k), split(v)
    s = np.einsum("bhqd,bhkd->bhqk", q, k) / np.sqrt(np.float32(HDl))
    s = s + attention_mask[:, None, :, :]
    s = s - s.max(axis=-1, keepdims=True)
    e = np.exp(s)
    w = e / e.sum(axis=-1, keepdims=True)
    o = np.einsum("bhqk,bhkd->bhqd", w, v)
    o = o.transpose(0, 2, 1, 3).reshape(B, S_, H_)
    return (o @ Wo + bo).astype(np.float32)


def kernel(query, key, value, attention_mask, Wq, bq, Wk, bk, Wv, bv, Wo, bo):
    query = np.asarray(query, np.float32)
    key = np.asarray(key, np.float32)
    value = np.asarray(value, np.float32)
    Wq, Wk, Wv, Wo = (np.asarray(a, np.float32) for a in (Wq, Wk, Wv, Wo))
    bq, bk, bv, bo = (np.asarray(a, np.float32) for a in (bq, bk, bv, bo))
    attention_mask = np.asarray(attention_mask, np.float32)

    if query.shape != (2, S, H) or Wq.shape != (H, H) or \
            attention_mask.shape != (2, S, S) or np.any(attention_mask):
        return _kernel_numpy(query, key, value, attention_mask,
                             Wq, bq, Wk, bk, Wv, bv, Wo, bo)

    qT = [np.ascontiguousarray(query[b].astype(ml_dtypes.bfloat16).T)
          for b in range(2)]
    kTh = [np.ascontiguousarray(key[b].astype(ml_dtypes.bfloat16).T)
           for b in range(2)]
    vTh = [np.ascontiguousarray(value[b].astype(ml_dtypes.bfloat16).T)
           for b in range(2)]

    nc = _get_nc()
    in_maps = []
    for c in range(N_CORES):
        b, hg = divmod(c, 4)
        sl = slice(D * hg, D * hg + D)
        in_maps.append({
            "xqT": qT[b],
            "xkT": kTh[b],
            "xvT": vTh[b],
            "wq": np.ascontiguousarray(Wq[:, sl]).astype(ml_dtypes.bfloat16),
            "wk": np.ascontiguousarray(Wk[:, sl]).astype(ml_dtypes.bfloat16),
            "wv": np.ascontiguousarray(Wv[:, sl]).astype(ml_dtypes.bfloat16),
            "wo": np.ascontiguousarray(Wo[sl, :]).astype(ml_dtypes.bfloat16),
            "bq2": bq[sl].reshape(2, 128).T.copy(),
            "bk2": bk[sl].reshape(2, 128).T.copy(),
            "bv1": bv[sl].reshape(1, D).copy(),
        })
    try:
        res = run_bass_kernel_spmd(nc, in_maps, list(range(N_CORES)))
    finally:
        # run_bass_via_pjrt monkeypatches libneuronxla.neuronx_cc; restore it
        # so later ordinary jax compiles in the caller's process are untouched.
        try:
            import libneuronxla  # pyright: ignore[reportMissingImports]
            if hasattr(libneuronxla, "orig_neuronx_cc"):
                libneuronxla.neuronx_cc = libneuronxla.orig_neuronx_cc
        except ImportError:
            pass
    outs = [res.results[c]["y"] for c in range(N_CORES)]
    out = np.empty((2, S, H), np.float32)
    for b in range(2):
        out[b] = (outs[4 * b].astype(np.float32)
                  + outs[4 * b + 1].astype(np.float32)
                  + outs[4 * b + 2].astype(np.float32)
                  + outs[4 * b + 3].astype(np.float32)) + bo
    return out


# revision 30
# speedup vs baseline: 1.0167x; 1.0167x over previous
"""Multi-head attention (B=2, S=2048, H=1024, NH=16) on 8 TRN2 NeuronCores.

Sharding: core c -> (batch b = c//4, head-group hg = c%4). Each core computes
Q/K/V projections for its 4 heads (256 columns of Wq/Wk/Wv), attention for
those heads, and a partial output projection (its 256 rows of Wo). Host sums
the 4 partials per batch and adds bo.

Per-core device pipeline (all matmuls at 1 cycle/row via bf16):
  - x is pre-transposed + bf16-cast on the host, so h-major xT streams
    straight into the projections.
  - Q/K projections produce qT/kT d-major [256, 2048] (W stationary);
    V s-major [2048, 4, 65] bf16 (xT stationary) with a ones column appended.
  - scoresT[sk, sq] per head pair: lhsT = kT head slice (K=64); the two
    heads land in the two PSUM banks of one [128, 2, 512] tile.
  - softmax exp runs on BOTH ACT and DVE concurrently: ACT exps head 0's
    bank (table exp, scale=1/8 fused); DVE exps head 1's bank with a custom
    microcoded op (deg-3 Horner + 2 squarings ~ exp(x/8), max rel err 1.7%
    at the +-3.6-sigma tails, 0.4% in the bulk -- end-to-end rel err 0.006
    vs 2e-2 budget). This doubles exp bandwidth so the PE never starves in
    the attention phase; pre-computed full tiles during the projection
    phase (on both engines) cover the remaining shortfall.
  - AV in [sq, d] orientation: lhsT = et 128-col chunk, rhs = v+ones
    [128, 65] -> psum [sq 128, 65]; col 64 accumulates the softmax
    denominators for free.
  - Normalization: batched DVE reciprocal + TensorScalarPtr multiply per
    (head, sq-chunk) writes normalized attn [sq, d] bf16 out of PSUM.
  - attn -> attnT via DMA XBAR transpose on the SP queue.
  - Output projection all-bf16: attnT stationary, Wo rows moving; PSUM
    drained by ACT copies (no bias add on device -- host adds bo), y DMA'd
    out per 128-row chunk, chunks spread across the following batch.
PSUM budget (8 banks): 2 proj/outproj + 2x2 score double-buffer + 2 AV.
"""
import os
import sys

if os.path.isdir("/opt/trn_rl_repo"):
    sys.path.insert(0, "/opt/trn_rl_repo")

from contextlib import ExitStack

import numpy as np
import ml_dtypes

import concourse.bass as bass
import concourse.tile as tile
from concourse import bacc, mybir
from concourse.bass import ts
from concourse.bass_utils import run_bass_kernel_spmd

F32 = mybir.dt.float32
F32R = mybir.dt.float32r
BF16 = mybir.dt.bfloat16
EXP = mybir.ActivationFunctionType.Exp
COPY = mybir.ActivationFunctionType.Copy

S = 2048
H = 1024
D = 256          # per-core head-slice width (4 heads x 64)
HD = 64
N_CORES = 8
SB = 512         # s-block
NSB = S // SB    # 4
HT = H // 128    # 8 h-tiles
SKT = S // 128   # 16 sk-tiles
SCALE = 1.0 / 8.0  # 1/sqrt(HD)

# deg-3 Horner coefficients for the DVE exp op, raw-score basis:
# exp(u/8) ~ (((1 + u*(EC0 + u*(EC1 + u*EC2)))^2)^2, |u/8| <= 3.6
EC0 = 0.031503140926361084
EC1 = 0.0005148400668986142
EC2 = 4.80940570923849e-06

_CACHE = {}


def _register_exp_op():
    """Define + register the custom DVE op EXP_P3SQSQ_ANT (idempotent).
    Registration appends to dve_ops.OPS at runtime so the per-NEFF DVE
    table generation and row assignment pick it up without editing the
    repo."""
    if "exp_op" in _CACHE:
        return _CACHE["exp_op"]
    import concourse.dve_ops as dvo
    from concourse.dve_spec import Spec, Src0, C0, C1, C2, One, sq, lower, \
        _has_src1
    from concourse.dve_uop import DveOpSpec

    name = "EXP_P3SQSQ_ANT"
    for o in dvo.OPS:
        if o.name == name:
            _CACHE["exp_op"] = o
            return o
    body = sq(sq(One + Src0 * (C0 + Src0 * (C1 + Src0 * C2))))

    def ref(in0, in1, c0, c1, c2):
        p = (1.0 + in0 * (c0 + in0 * (c1 + in0 * c2))).astype(np.float32)
        p = (p * p).astype(np.float32)
        return (p * p).astype(np.float32)

    spec = Spec(body=body, reference=ref)
    row = dvo._CUSTOM_DVE_ROW_BASE + len(dvo.OPS)
    assert row < 0x20
    shas = {}
    for ver in ("v3", "v4"):
        try:
            uops = lower(spec, ver=ver)
            shas[ver] = DveOpSpec(name=name, opcode=row, uops=uops,
                                  rd1_en=_has_src1(spec)).sha(ver)
        except Exception:
            if ver == "v3":
                raise
    op = dvo.DveOp(name, spec, subdim=False, uops_sha=shas)
    dvo.OPS.append(op)
    dvo._SUB_OPCODE_FOR_NAME[name] = row
    dvo.CUSTOM_DVE_SPECS[name] = spec
    _CACHE["exp_op"] = op
    return op


def _build():
    exp_op = _register_exp_op()
    nc = bacc.Bacc("TRN2", target_bir_lowering=False, debug=False,
                   num_devices=N_CORES)

    xq = nc.dram_tensor("xqT", [H, S], BF16, kind="ExternalInput").ap()
    xk = nc.dram_tensor("xkT", [H, S], BF16, kind="ExternalInput").ap()
    xv = nc.dram_tensor("xvT", [H, S], BF16, kind="ExternalInput").ap()
    wq_d = nc.dram_tensor("wq", [H, D], BF16, kind="ExternalInput").ap()
    wk_d = nc.dram_tensor("wk", [H, D], BF16, kind="ExternalInput").ap()
    wv_d = nc.dram_tensor("wv", [H, D], BF16, kind="ExternalInput").ap()
    wo_d = nc.dram_tensor("wo", [D, H], BF16, kind="ExternalInput").ap()
    bq_d = nc.dram_tensor("bq2", [128, 2], F32, kind="ExternalInput").ap()
    bk_d = nc.dram_tensor("bk2", [128, 2], F32, kind="ExternalInput").ap()
    bv_d = nc.dram_tensor("bv1", [1, D], F32, kind="ExternalInput").ap()
    y = nc.dram_tensor("y", [S, H], BF16, kind="ExternalOutput").ap()

    def dve_exp(out_ap, in_ap):
        nc.vector._custom_dve(exp_op, out=out_ap, in0=in_ap,
                              s0=EC0, s1=EC1, imm2=EC2)

    with tile.TileContext(nc) as tc:
        with ExitStack() as ctx:
            const = ctx.enter_context(tc.tile_pool(name="const", bufs=1))
            pers = ctx.enter_context(tc.tile_pool(name="pers", bufs=1))
            xt_p = ctx.enter_context(tc.tile_pool(name="xt", bufs=2))
            small = ctx.enter_context(tc.tile_pool(name="small", bufs=4))
            pre_p = ctx.enter_context(tc.tile_pool(name="prep", bufs=94))
            eh_p = ctx.enter_context(tc.tile_pool(name="ehp", bufs=10))
            atn_p = ctx.enter_context(tc.tile_pool(name="atnp", bufs=6))
            fin_p = ctx.enter_context(tc.tile_pool(name="finp", bufs=4))

            # ---- constants ----
            # weights + biases go out on the ACT HWDGE queue so their
            # descriptor generation runs in parallel with the SP queue's
            # xt streams (two HWDGEs).
            wq = const.tile([128, HT, D], BF16)
            wq_r = wq_d.rearrange("(j p) d -> p j d", p=128)
            nc.scalar.dma_start(wq[:, :, 0:128], wq_r[:, :, 0:128])
            wk = const.tile([128, HT, D], BF16)
            wv = const.tile([128, HT, D], BF16)
            bq2 = const.tile([128, 2], F32)
            bk2 = const.tile([128, 2], F32)
            bv1 = const.tile([1, D], F32)
            ones_f = const.tile([1, 128], F32)
            nc.gpsimd.memset(ones_f[:], 1.0)
            ones = const.tile([1, 128], F32R)
            nc.vector.tensor_copy(ones[:], ones_f[:])
            bv1r = const.tile([1, D], F32R)
            warm = const.tile([1, 2], BF16)
            nc.scalar.activation(warm[:], ones_f[0:1, 0:2], EXP)
            # [128,128] bf16 identity for the tail's PE transposes
            id1 = const.tile([128, 128], BF16)
            nc.gpsimd.memset(id1[:], 1.0)
            ident = const.tile([128, 128], BF16)
            nc.gpsimd.affine_select(ident[:], id1[:], pattern=[[1, 128]],
                                    compare_op=mybir.AluOpType.is_equal,
                                    fill=0.0, base=0, channel_multiplier=-1)

            # ---- persistent activations ----
            qT = pers.tile([128, 2, S], BF16)   # [d_local, dh, s]
            kT = pers.tile([128, 2, S], BF16)
            vS = pers.tile([128, SKT, 4, HD + 1], BF16)  # [sk, skt, head, d|1]
            nc.gpsimd.memset(vS[:], 1.0)       # ones column (rest overwritten)
            attnT = pers.tile([128, 2, S], BF16)  # [d in pair, hp, sq]

            ps_pj = ctx.enter_context(
                tc.tile_pool(name="ps_pj", bufs=2, space="PSUM"))
            ps_qk = ctx.enter_context(
                tc.tile_pool(name="ps_qk", bufs=4, space="PSUM"))
            ps_av = ctx.enter_context(
                tc.tile_pool(name="ps_av", bufs=2, space="PSUM"))

            bvb = const.tile([128, D], F32)

            def load_xt(xd, sb, name):
                """DMA one s-block of pre-transposed x: [128h, HT, SB] bf16."""
                xt = xt_p.tile([128, HT, SB], BF16, tag="xt", name=name)
                nc.sync.dma_start(
                    xt[:], xd.rearrange("(j p) s -> p j s", p=128)[
                        :, :, ts(sb, SB)])
                return xt

            def proj_dmajor_unit(xt, w, bias2, dst, sb, dh, c0=0, c1=SB):
                # dst[:, dh, sb*SB+c0:+c1] = (x @ w + b).T (d-major)
                pp = ps_pj.tile([128, 512], F32, tag="pj", name="pp")
                for j in range(HT):
                    nc.tensor.matmul(pp[:, 0:c1 - c0], w[:, j, ts(dh, 128)],
                                     xt[:, j, c0:c1],
                                     start=(j == 0), stop=(j == HT - 1))
                nc.vector.tensor_scalar_add(
                    dst[:, dh, sb * SB + c0:sb * SB + c1], pp[:, 0:c1 - c0],
                    bias2[:, dh:dh + 1])

            def qk_score_h(hp, sqb, sk, hh):
                # one head's [128 sk, 512 sq] score tile: a single PSUM
                # bank, so the 4-deep ring gives two full tiles of exp
                # lookahead (the exp+semaphore round trip is ~1us while the
                # PE's per-tile work is ~0.65us).
                pqk = ps_qk.tile([128, 512], F32, tag="qk", name="pqk")
                r0 = HD * hh
                nc.tensor.matmul(
                    pqk[:],
                    kT[r0:r0 + HD, hp, ts(sk, 128)],
                    qT[r0:r0 + HD, hp, ts(sqb, SB)],
                    start=True, stop=True)
                return pqk

            def qk_exp_pre(hp, sqb, sk, eng="aa"):
                # pre-tile path (projection phase): per-head tiles, engine
                # per half given by `eng` (a=ACT, d=DVE).
                ets = []
                for hh in range(2):
                    pqk = qk_score_h(hp, sqb, sk, hh)
                    et = pre_p.tile([128, 512], BF16, tag="e", name="et")
                    if eng[hh] == "a":
                        nc.scalar.activation(et[:], pqk[:], EXP, scale=SCALE)
                    else:
                        dve_exp(et[:], pqk[:])
                    ets.append(et)
                return tuple(ets)

            def qk_exp_split(hp, sqb, sk):
                # inline path: head 0 -> ACT, head 1 -> DVE, concurrently.
                pqk0 = qk_score_h(hp, sqb, sk, 0)
                et0 = eh_p.tile([128, 512], BF16, tag="eh", name="et0")
                nc.scalar.activation(et0[:], pqk0[:], EXP, scale=SCALE)
                pqk1 = qk_score_h(hp, sqb, sk, 1)
                et1 = eh_p.tile([128, 512], BF16, tag="eh", name="et1")
                dve_exp(et1[:], pqk1[:])
                return (et0, et1)

            def av_accum(hp, sk, et, pav):
                # pav[hh][:, sqc, 0:65] += et[hh][:,chunk].T @ v+ones
                # start=True marks the whole 2KB PSUM bank pending-zero, so
                # only the bank's first group may use it; later groups'
                # first accumulate reads pending-zero bytes as zero.
                for hh in range(2):
                    for sqc in range(4):
                        nc.tensor.matmul(
                            pav[hh][:, sqc, 0:HD + 1],
                            et[hh][:, ts(sqc, 128)],
                            vS[:, sk, 2 * hp + hh, :],
                            start=(sk == 0 and sqc == 0),
                            stop=(sk == SKT - 1),
                            skip_group_check=True)

            def emit_outproj_mm(sqb, st):
                # po matmuls only; the ACT drain copies + y DMA are emitted
                # later (emit_outproj_drain) so they sit BEHIND the next exp
                # in the in-order ACT stream instead of blocking it.
                fin = fin_p.tile([128, H], BF16, tag="fin", name="fin")
                pos = []
                for eb in range(2):
                    po = ps_pj.tile([128, 512], F32, tag="pj", name="po")
                    nc.tensor.matmul(po[:],
                                     attnT[:, 0, ts(4 * sqb + st, 128)],
                                     wo[:, 0, ts(eb, 512)],
                                     start=True, stop=False,
                                     skip_group_check=True)
                    nc.tensor.matmul(po[:],
                                     attnT[:, 1, ts(4 * sqb + st, 128)],
                                     wo[:, 1, ts(eb, 512)],
                                     start=False, stop=True,
                                     skip_group_check=True)
                    pos.append(po)
                return (sqb, st, fin, pos)

            def emit_outproj_drain(w):
                # one PSUM->SBUF copy per engine so neither in-order queue
                # eats both
                sqb, st, fin, pos = w
                nc.scalar.activation(fin[:, ts(0, 512)], pos[0][:], COPY)
                nc.vector.tensor_copy(fin[:, ts(1, 512)], pos[1][:])
                nc.sync.dma_start(y[ts(4 * sqb + st, 128), :], fin[:])

            def attn_finish(hp, sqb, pav, fuse_outproj=False):
                # normalize out of PSUM (per-partition denominators in col 64)
                # and transpose [sq, d] -> attnT[d, sq] on the DMA XBAR (SP).
                # The per-chunk scale multiplies are split across DVE
                # (tensor_scalar) and ACT (Copy with per-partition scale) to
                # halve the batch-boundary normalize latency.
                atn = [atn_p.tile([128, 128], BF16, tag="atn", name="atn")
                       for _ in range(4)]
                recs = []
                for hh in range(2):
                    rec = small.tile([128, 4], F32, tag="rec", name="rec")
                    with nc.allow_low_precision(reason="softmax denom recip"):
                        nc.vector.reciprocal(rec[:], pav[hh][:, :, HD:HD + 1])
                    recs.append(rec)
                # tail fast-drain for the last batch: every chunk's head-0
                # outproj mm runs DURING the normalize+transpose window (its
                # attnT half landed a batch ago), on PSUM borrowed from the
                # now-idle qk/pj rings; chunk 3 takes the av banks once the
                # normalize reads drain. Chunk k then completes (head-1 mm +
                # drain + y DMA) as its transpose lands. Transposes alternate
                # SP/ACT queues so their configs don't serialize.
                tails = []
                if fuse_outproj:
                    for st in range(2):
                        fin = fin_p.tile([128, H], BF16, tag="fin",
                                         name="fin")
                        pos = []
                        for eb in range(2):
                            po = ps_qk.tile([128, 512], F32, tag="qk",
                                            name="pot")
                            nc.tensor.matmul(
                                po[:], attnT[:, 0, ts(4 * sqb + st, 128)],
                                wo[:, 0, ts(eb, 512)], start=True,
                                stop=False, skip_group_check=True)
                            pos.append(po)
                        tails.append((st, fin, pos))
                for sqc in range(4):
                    nc.vector.tensor_scalar_mul(
                        atn[sqc][:, ts(0, HD)],
                        pav[0][:, sqc, 0:HD],
                        recs[0][:, sqc:sqc + 1])
                    nc.scalar.activation(
                        atn[sqc][:, ts(1, HD)],
                        pav[1][:, sqc, 0:HD], COPY,
                        scale=recs[1][:, sqc:sqc + 1])
                    if not fuse_outproj:
                        nc.sync.dma_start_transpose(
                            attnT[:, hp, ts(4 * sqb + sqc, 128)],
                            atn[sqc][:])
                    else:
                        # tail: transpose on the PE (53ns + a short copy)
                        # instead of the ~2.3us DMA XBAR round trip; copies
                        # alternate DVE/ACT.
                        ptr = ps_pj.tile([128, 128], BF16, tag="pj",
                                         name="ptr")
                        nc.tensor.matmul(ptr[:], atn[sqc][:], ident[:],
                                         is_transpose=True,
                                         skip_group_check=True)
                        dst = attnT[:, hp, ts(4 * sqb + sqc, 128)]
                        if sqc % 2:
                            nc.scalar.activation(dst, ptr[:], COPY)
                        else:
                            nc.vector.tensor_copy(dst, ptr[:])
                if fuse_outproj:
                    # chunk 3's head-0 mms go to the av banks -- emitted
                    # after the muls above so the WAR on the freshly-read
                    # pav banks is tracked; chunk 2 reuses the pj ring after
                    # the transpose copies drain it.
                    for st in (3, 2):
                        fin = fin_p.tile([128, H], BF16, tag="fin",
                                         name="fin")
                        pos = []
                        for eb in range(2):
                            if st == 3:
                                pot = ps_av.tile([128, 4, 128], F32,
                                                 tag="av", name="pot")
                                po = pot.rearrange("p a b -> p (a b)")
                            else:
                                po = ps_pj.tile([128, 512], F32, tag="pj",
                                                name="pot")
                            nc.tensor.matmul(
                                po[:], attnT[:, 0, ts(4 * sqb + st, 128)],
                                wo[:, 0, ts(eb, 512)], start=True,
                                stop=False, skip_group_check=True)
                            pos.append(po)
                        tails.append((st, fin, pos))
                    tails.sort()
                    for st, fin, pos in tails:
                        for eb in range(2):
                            nc.tensor.matmul(
                                pos[eb][:],
                                attnT[:, 1, ts(4 * sqb + st, 128)],
                                wo[:, 1, ts(eb, 512)], start=False,
                                stop=True, skip_group_check=True)
                        nc.scalar.activation(fin[:, ts(0, 512)], pos[0][:],
                                             COPY)
                        nc.vector.tensor_copy(fin[:, ts(1, 512)], pos[1][:])
                        nc.sync.dma_start(y[ts(4 * sqb + st, 128), :],
                                          fin[:])

            # ---- streaming loads + projections, with scores+exp for ready
            # (sqb, hp, sk) tiles pre-emitted in consumption order so both
            # exp engines start chewing softmax work early. ----
            def spread(p):
                return sorted({int(round(i * SKT / p)) for i in range(p)})

            # pre-tiles: ACT-only (a DVE pre-exp would head-of-line-block
            # the projection epilogue adds on the in-order DVE queue and
            # stall the PE's ps_pj ring). Spread within each batch so the
            # inline ACT/DVE load stays even through the attention stream.
            # leading + trailing sks per batch: a pre-covered batch END lets
            # the engine queues drain before the normalize, so the boundary
            # WAR on the pav ring resolves fast; a pre-covered batch START
            # gives the PE immediate AV work after it. b00 (consumed inside
            # the projection phase) stays ACT-only; later pre tiles put one
            # half on DVE -- at most one 0.66us DVE op lands between
            # projection epilogue adds, within the pp ring's 2-unit slack.
            PRE_SPEC = [((0, 0), list(range(SKT)), "aa"),
                        ((0, 1), [0, 1, 13, 14, 15], "ad"),
                        ((1, 0), [0, 12, 13, 14, 15], "ad"),
                        ((1, 1), [0, 12, 13, 14, 15], "ad"),
                        ((2, 0), [0, 13, 14, 15], "ad"),
                        ((2, 1), [0, 13, 14, 15], "ad"),
                        ((3, 0), [0, 14, 15], "ad")]
            pre_order = []
            for (sqb, hp), sks, eng in PRE_SPEC:
                for sk in sks:
                    pre_order.append((sqb, hp, sk, eng))
            pre = {}
            st_pre = {"i": 0, "q": set(), "k": set()}

            def emit_pre(limit):
                # dh-granular readiness: head-pair hp only needs the dh=hp
                # halves of its qT/kT blocks.
                done = 0
                while st_pre["i"] < len(pre_order) and done < limit:
                    sqb, hp, sk, eng = pre_order[st_pre["i"]]
                    if (sqb, hp) not in st_pre["q"] or \
                            (sk // 4, hp) not in st_pre["k"]:
                        break
                    pre[(sqb, hp, sk)] = qk_exp_pre(hp, sqb, sk, eng)
                    st_pre["i"] += 1
                    done += 1

            def proj_qk(xd, w, bias2, dst, sb, which):
                xt = load_xt(xd, sb, "xt" + which)
                for dh in range(2):
                    proj_dmajor_unit(xt, w, bias2, dst, sb, dh)
                    st_pre[which].add((sb, dh))
                    emit_pre(3)

            # batch (0,0)'s AV interleaves into the V phase: its et tiles
            # are all precomputed, and vS[sk] is ready right after block
            # sk//4's epilogue -- so the first attention batch costs no
            # wall-clock of its own.
            pav00 = [ps_av.tile([128, 4, 128], F32, tag="av", name=f"pav{hh}")
                     for hh in range(2)]

            def proj_v(sb):
                # batch (0,0)'s AV trails the V epilogues by one si unit so
                # the in-order PE never waits on the DVE vS write latency.
                xtv = load_xt(xv, sb, "xtv")
                for si in range(4):
                    pv = ps_pj.tile([128, 512], F32, tag="pj", name="pv")
                    for j in range(HT):
                        nc.tensor.matmul(pv[:, 0:D],
                                         xtv[:, j, ts(si, 128)],
                                         wv[:, j, :],
                                         start=(j == 0), stop=(j == HT - 1))
                    nc.vector.tensor_add(
                        vS[:, 4 * sb + si, :, 0:HD],
                        pv[:, 0:D].rearrange("p (g d) -> p g d", g=4),
                        bvb[:].rearrange("p (g d) -> p g d", g=4))
                    emit_pre(3)
                    sk = 4 * sb + si
                    if sk > 0:
                        av_accum(0, sk - 1, pre.pop((0, 0, sk - 1)), pav00)

            # Q0 then all K (unlocks every sqb0 tile), then Q1-3 (unlocks
            # the rest), V last (first consumed only once attention starts).
            # xtq0 lands in two halves so the first projection matmul can
            # start on the first 256 columns while the rest streams in.
            xtq0 = xt_p.tile([128, HT, SB], BF16, tag="xt", name="xtq0")
            xq_r = xq.rearrange("(j p) s -> p j s", p=128)
            nc.sync.dma_start(xtq0[:, :, 0:256], xq_r[:, :, 0:256])
            nc.sync.dma_start(xtq0[:, :, 256:512], xq_r[:, :, 256:512])
            nc.scalar.dma_start(bq2[:], bq_d[:])
            wk_r = wk_d.rearrange("(j p) d -> p j d", p=128)
            nc.scalar.dma_start(wk[:, :, 0:128], wk_r[:, :, 0:128])
            # fast start: narrow first xk load + mini K projection puts the
            # first score+exp on the engines early.
            xtk0a = xt_p.tile([128, HT, 128], BF16, tag="xta", name="xtk0a", bufs=1)
            nc.sync.dma_start(wq[:, :, 128:256], wq_r[:, :, 128:256])
            nc.sync.dma_start(
                xtk0a[:], xk.rearrange("(j p) s -> p j s", p=128)[:, :, 0:128])
            nc.scalar.dma_start(bk2[:], bk_d[:])
            nc.scalar.dma_start(wk[:, :, 128:256], wk_r[:, :, 128:256])
            proj_dmajor_unit(xtq0, wq, bq2, qT, 0, 0, 0, 256)
            proj_dmajor_unit(xtq0, wq, bq2, qT, 0, 0, 256, 512)
            st_pre["q"].add((0, 0))
            pk0 = ps_pj.tile([128, 512], F32, tag="pj", name="pk0")
            for j in range(HT):
                nc.tensor.matmul(pk0[:, 0:128], wk[:, j, 0:128],
                                 xtk0a[:, j, :],
                                 start=(j == 0), stop=(j == HT - 1))
            nc.vector.tensor_scalar_add(kT[:, 0, 0:128], pk0[:, 0:128],
                                        bk2[:, 0:1])
            pre[(0, 0, 0)] = qk_exp_pre(0, 0, 0, "aa")
            proj_dmajor_unit(xtq0, wq, bq2, qT, 0, 1)
            st_pre["q"].add((0, 1))
            st_pre["i"] = 1
            xtk0 = load_xt(xk, 0, "xtk")
            proj_dmajor_unit(xtk0, wk, bk2, kT, 0, 0, 128, SB)
            st_pre["k"].add((0, 0))
            emit_pre(3)
            proj_dmajor_unit(xtk0, wk, bk2, kT, 0, 1)
            st_pre["k"].add((0, 1))
            emit_pre(3)
            for sb in range(1, NSB):
                proj_qk(xk, wk, bk2, kT, sb, "k")
                emit_pre(3)
            for sb in range(1, NSB):
                proj_qk(xq, wq, bq2, qT, sb, "q")
                emit_pre(3)
            nc.sync.dma_start(wv[:], wv_d.rearrange("(j p) d -> p j d", p=128))
            # v-bias broadcast, deferred here so its small DMAs stay off the
            # critical startup path (first needed by V0's epilogue)
            nc.sync.dma_start(bv1[:], bv_d[:])
            nc.vector.tensor_copy(bv1r[:], bv1[:])
            pbc = ps_pj.tile([128, 512], F32, tag="pj", name="pbc")
            nc.tensor.matmul(pbc[:, 0:D], ones[0:1, :], bv1r[:])
            nc.vector.tensor_copy(bvb[:], pbc[:, 0:D])
            for sb in range(NSB):
                proj_v(sb)
            av_accum(0, SKT - 1, pre.pop((0, 0, SKT - 1)), pav00)
            emit_pre(len(pre_order))

            # deferred: output-projection weights (first needed ~60us in)
            wo = const.tile([128, 2, H], BF16)
            nc.sync.dma_start(wo[:], wo_d.rearrange("(i p) e -> p i e", p=128))

            # ---- attention stream. Inline tiles split each head pair's
            # exp across ACT/DVE (separate PSUM banks), with 2-item score
            # lookahead across batch boundaries. outproj(sqb-1) chunks are
            # spread through the following batch (sk = 2,5,8,11) so the
            # ps_pj ring never backs up on the ACT drain. ----
            attn_finish(0, 0, pav00)
            batches = [(s, h) for s in range(NSB) for h in range(2)][1:]
            stream = [(s, h, k) for (s, h) in batches for k in range(SKT)]
            emitted = {}
            st_la = {"ep": 0}

            def pump(upto, inline_ahead=0):
                # emit score+exp for stream items <= upto, plus keep
                # `inline_ahead` INLINE (non-pre) tiles in flight beyond
                # the consumer. `emitted` holds exactly the un-consumed
                # inline tiles, so len(emitted) IS the in-flight count --
                # pre-covered items don't eat the lookahead window.
                while st_la["ep"] < len(stream) and \
                        (st_la["ep"] <= upto or len(emitted) < inline_ahead):
                    key = stream[st_la["ep"]]
                    if key not in pre:
                        emitted[key] = qk_exp_split(key[1], key[0], key[2])
                    st_la["ep"] += 1

            deferred = []
            pending_drain = []
            pav_cur = {}
            for ci, key in enumerate(stream):
                sqb, hp, sk = key
                if sk == 0:
                    pav_cur[(sqb, hp)] = [
                        ps_av.tile([128, 4, 128], F32, tag="av",
                                   name=f"pav{hh}") for hh in range(2)]
                # ensure the current tile's exp exists; the lookahead pump
                # at loop end runs AFTER any finish so normalize ops aren't
                # queued behind the next batch's exps.
                pump(ci)
                if pending_drain:
                    emit_outproj_drain(pending_drain.pop(0))
                et = pre.pop(key, None)
                if et is None:
                    et = emitted.pop(key)
                av_accum(hp, sk, et, pav_cur[(sqb, hp)])
                if deferred and sk in (3, 6, 9, 12):
                    dq, dst = deferred[0]
                    pending_drain.append(emit_outproj_mm(dq, dst))
                    if dst == 3:
                        deferred.pop(0)
                    else:
                        deferred[0] = (dq, dst + 1)
                if sk == SKT - 1:
                    last = (sqb == NSB - 1 and hp == 1)
                    attn_finish(hp, sqb, pav_cur.pop((sqb, hp)),
                                fuse_outproj=last)
                    if hp == 0 and sqb > 0:
                        deferred.append((sqb - 1, 0))
                pump(ci, inline_ahead=2)

    nc.compile()
    return nc


def _get_nc():
    if "nc" not in _CACHE:
        _CACHE["nc"] = _build()
    return _CACHE["nc"]


def _kernel_numpy(query, key, value, attention_mask,
                  Wq, bq, Wk, bk, Wv, bv, Wo, bo):
    """Exact fp32 numpy fallback (only used for inputs outside the spec:
    nonzero mask or unexpected shapes)."""
    B, S_, H_ = query.shape
    NH = 16
    HDl = H_ // NH
    q = query @ Wq + bq
    k = key @ Wk + bk
    v = value @ Wv + bv

    def split(x):
        return x.reshape(B, S_, NH, HDl).transpose(0, 2, 1, 3)

    q, k, v = split(q), split(k), split(v)
    s = np.einsum("bhqd,bhkd->bhqk", q, k) / np.sqrt(np.float32(HDl))
    s = s + attention_mask[:, None, :, :]
    s = s - s.max(axis=-1, keepdims=True)
    e = np.exp(s)
    w = e / e.sum(axis=-1, keepdims=True)
    o = np.einsum("bhqk,bhkd->bhqd", w, v)
    o = o.transpose(0, 2, 1, 3).reshape(B, S_, H_)
    return (o @ Wo + bo).astype(np.float32)


def kernel(query, key, value, attention_mask, Wq, bq, Wk, bk, Wv, bv, Wo, bo):
    query = np.asarray(query, np.float32)
    key = np.asarray(key, np.float32)
    value = np.asarray(value, np.float32)
    Wq, Wk, Wv, Wo = (np.asarray(a, np.float32) for a in (Wq, Wk, Wv, Wo))
    bq, bk, bv, bo = (np.asarray(a, np.float32) for a in (bq, bk, bv, bo))
    attention_mask = np.asarray(attention_mask, np.float32)

    if query.shape != (2, S, H) or Wq.shape != (H, H) or \
            attention_mask.shape != (2, S, S) or np.any(attention_mask):
        return _kernel_numpy(query, key, value, attention_mask,
                             Wq, bq, Wk, bk, Wv, bv, Wo, bo)

    qT = [np.ascontiguousarray(query[b].astype(ml_dtypes.bfloat16).T)
          for b in range(2)]
    kTh = [np.ascontiguousarray(key[b].astype(ml_dtypes.bfloat16).T)
           for b in range(2)]
    vTh = [np.ascontiguousarray(value[b].astype(ml_dtypes.bfloat16).T)
           for b in range(2)]

    nc = _get_nc()
    in_maps = []
    for c in range(N_CORES):
        b, hg = divmod(c, 4)
        sl = slice(D * hg, D * hg + D)
        in_maps.append({
            "xqT": qT[b],
            "xkT": kTh[b],
            "xvT": vTh[b],
            "wq": np.ascontiguousarray(Wq[:, sl]).astype(ml_dtypes.bfloat16),
            "wk": np.ascontiguousarray(Wk[:, sl]).astype(ml_dtypes.bfloat16),
            "wv": np.ascontiguousarray(Wv[:, sl]).astype(ml_dtypes.bfloat16),
            "wo": np.ascontiguousarray(Wo[sl, :]).astype(ml_dtypes.bfloat16),
            "bq2": bq[sl].reshape(2, 128).T.copy(),
            "bk2": bk[sl].reshape(2, 128).T.copy(),
            "bv1": bv[sl].reshape(1, D).copy(),
        })
    try:
        res = run_bass_kernel_spmd(nc, in_maps, list(range(N_CORES)))
    finally:
        # run_bass_via_pjrt monkeypatches libneuronxla.neuronx_cc; restore it
        # so later ordinary jax compiles in the caller's process are untouched.
        try:
            import libneuronxla  # pyright: ignore[reportMissingImports]
            if hasattr(libneuronxla, "orig_neuronx_cc"):
                libneuronxla.neuronx_cc = libneuronxla.orig_neuronx_cc
        except ImportError:
            pass
    outs = [res.results[c]["y"] for c in range(N_CORES)]
    out = np.empty((2, S, H), np.float32)
    for b in range(2):
        out[b] = (outs[4 * b].astype(np.float32)
                  + outs[4 * b + 1].astype(np.float32)
                  + outs[4 * b + 2].astype(np.float32)
                  + outs[4 * b + 3].astype(np.float32)) + bo
    return out


# revision 31
# speedup vs baseline: 1.0201x; 1.0034x over previous
"""Multi-head attention (B=2, S=2048, H=1024, NH=16) on 8 TRN2 NeuronCores.

Sharding: core c -> (batch b = c//4, head-group hg = c%4). Each core computes
Q/K/V projections for its 4 heads (256 columns of Wq/Wk/Wv), attention for
those heads, and a partial output projection (its 256 rows of Wo). Host sums
the 4 partials per batch and adds bo.

Per-core device pipeline (all matmuls at 1 cycle/row via bf16):
  - x is pre-transposed + bf16-cast on the host, so h-major xT streams
    straight into the projections.
  - Q/K projections produce qT/kT d-major [256, 2048] (W stationary);
    V s-major [2048, 4, 65] bf16 (xT stationary) with a ones column appended.
  - scoresT[sk, sq] per head pair: lhsT = kT head slice (K=64); the two
    heads land in the two PSUM banks of one [128, 2, 512] tile.
  - softmax exp runs on BOTH ACT and DVE concurrently: ACT exps head 0's
    bank (table exp, scale=1/8 fused); DVE exps head 1's bank with a custom
    microcoded op (deg-3 Horner + 2 squarings ~ exp(x/8), max rel err 1.7%
    at the +-3.6-sigma tails, 0.4% in the bulk -- end-to-end rel err 0.006
    vs 2e-2 budget). This doubles exp bandwidth so the PE never starves in
    the attention phase; pre-computed full tiles during the projection
    phase (on both engines) cover the remaining shortfall.
  - AV in [sq, d] orientation: lhsT = et 128-col chunk, rhs = v+ones
    [128, 65] -> psum [sq 128, 65]; col 64 accumulates the softmax
    denominators for free.
  - Normalization: batched DVE reciprocal + TensorScalarPtr multiply per
    (head, sq-chunk) writes normalized attn [sq, d] bf16 out of PSUM.
  - attn -> attnT via DMA XBAR transpose on the SP queue.
  - Output projection all-bf16: attnT stationary, Wo rows moving; PSUM
    drained by ACT copies (no bias add on device -- host adds bo), y DMA'd
    out per 128-row chunk, chunks spread across the following batch.
PSUM budget (8 banks): 2 proj/outproj + 2x2 score double-buffer + 2 AV.
"""
import os
import sys

if os.path.isdir("/opt/trn_rl_repo"):
    sys.path.insert(0, "/opt/trn_rl_repo")

from contextlib import ExitStack

import numpy as np
import ml_dtypes

import concourse.bass as bass
import concourse.tile as tile
from concourse import bacc, mybir
from concourse.bass import ts
from concourse.bass_utils import run_bass_kernel_spmd

F32 = mybir.dt.float32
F32R = mybir.dt.float32r
BF16 = mybir.dt.bfloat16
EXP = mybir.ActivationFunctionType.Exp
COPY = mybir.ActivationFunctionType.Copy

S = 2048
H = 1024
D = 256          # per-core head-slice width (4 heads x 64)
HD = 64
N_CORES = 8
SB = 512         # s-block
NSB = S // SB    # 4
HT = H // 128    # 8 h-tiles
SKT = S // 128   # 16 sk-tiles
SCALE = 1.0 / 8.0  # 1/sqrt(HD)

# deg-3 Horner coefficients for the DVE exp op, raw-score basis:
# exp(u/8) ~ (((1 + u*(EC0 + u*(EC1 + u*EC2)))^2)^2, |u/8| <= 3.6
EC0 = 0.031503140926361084
EC1 = 0.0005148400668986142
EC2 = 4.80940570923849e-06

_CACHE = {}


def _register_exp_op():
    """Define + register the custom DVE op EXP_P3SQSQ_ANT (idempotent).
    Registration appends to dve_ops.OPS at runtime so the per-NEFF DVE
    table generation and row assignment pick it up without editing the
    repo."""
    if "exp_op" in _CACHE:
        return _CACHE["exp_op"]
    import concourse.dve_ops as dvo
    from concourse.dve_spec import Spec, Src0, C0, C1, C2, One, sq, lower, \
        _has_src1
    from concourse.dve_uop import DveOpSpec

    name = "EXP_P3SQSQ_ANT"
    for o in dvo.OPS:
        if o.name == name:
            _CACHE["exp_op"] = o
            return o
    body = sq(sq(One + Src0 * (C0 + Src0 * (C1 + Src0 * C2))))

    def ref(in0, in1, c0, c1, c2):
        p = (1.0 + in0 * (c0 + in0 * (c1 + in0 * c2))).astype(np.float32)
        p = (p * p).astype(np.float32)
        return (p * p).astype(np.float32)

    spec = Spec(body=body, reference=ref)
    row = dvo._CUSTOM_DVE_ROW_BASE + len(dvo.OPS)
    assert row < 0x20
    shas = {}
    for ver in ("v3", "v4"):
        try:
            uops = lower(spec, ver=ver)
            shas[ver] = DveOpSpec(name=name, opcode=row, uops=uops,
                                  rd1_en=_has_src1(spec)).sha(ver)
        except Exception:
            if ver == "v3":
                raise
    op = dvo.DveOp(name, spec, subdim=False, uops_sha=shas)
    dvo.OPS.append(op)
    dvo._SUB_OPCODE_FOR_NAME[name] = row
    dvo.CUSTOM_DVE_SPECS[name] = spec
    _CACHE["exp_op"] = op
    return op


def _build():
    exp_op = _register_exp_op()
    nc = bacc.Bacc("TRN2", target_bir_lowering=False, debug=False,
                   num_devices=N_CORES)

    xq = nc.dram_tensor("xqT", [H, S], BF16, kind="ExternalInput").ap()
    xk = nc.dram_tensor("xkT", [H, S], BF16, kind="ExternalInput").ap()
    xv = nc.dram_tensor("xvT", [H, S], BF16, kind="ExternalInput").ap()
    wq_d = nc.dram_tensor("wq", [H, D], BF16, kind="ExternalInput").ap()
    wk_d = nc.dram_tensor("wk", [H, D], BF16, kind="ExternalInput").ap()
    wv_d = nc.dram_tensor("wv", [H, D], BF16, kind="ExternalInput").ap()
    wo_d = nc.dram_tensor("wo", [D, H], BF16, kind="ExternalInput").ap()
    bq_d = nc.dram_tensor("bq2", [128, 2], F32, kind="ExternalInput").ap()
    bk_d = nc.dram_tensor("bk2", [128, 2], F32, kind="ExternalInput").ap()
    bv_d = nc.dram_tensor("bv1", [1, D], F32, kind="ExternalInput").ap()
    y = nc.dram_tensor("y", [S, H], BF16, kind="ExternalOutput").ap()

    def dve_exp(out_ap, in_ap):
        nc.vector._custom_dve(exp_op, out=out_ap, in0=in_ap,
                              s0=EC0, s1=EC1, imm2=EC2)

    with tile.TileContext(nc) as tc:
        with ExitStack() as ctx:
            const = ctx.enter_context(tc.tile_pool(name="const", bufs=1))
            pers = ctx.enter_context(tc.tile_pool(name="pers", bufs=1))
            xt_p = ctx.enter_context(tc.tile_pool(name="xt", bufs=2))
            small = ctx.enter_context(tc.tile_pool(name="small", bufs=4))
            pre_p = ctx.enter_context(tc.tile_pool(name="prep", bufs=94))
            eh_p = ctx.enter_context(tc.tile_pool(name="ehp", bufs=10))
            atn_p = ctx.enter_context(tc.tile_pool(name="atnp", bufs=6))
            fin_p = ctx.enter_context(tc.tile_pool(name="finp", bufs=4))

            # ---- constants ----
            # weights + biases go out on the ACT HWDGE queue so their
            # descriptor generation runs in parallel with the SP queue's
            # xt streams (two HWDGEs).
            wq = const.tile([128, HT, D], BF16)
            wq_r = wq_d.rearrange("(j p) d -> p j d", p=128)
            nc.scalar.dma_start(wq[:, :, 0:128], wq_r[:, :, 0:128])
            wk = const.tile([128, HT, D], BF16)
            wv = const.tile([128, HT, D], BF16)
            bq2 = const.tile([128, 2], F32)
            bk2 = const.tile([128, 2], F32)
            bv1 = const.tile([1, D], F32)
            ones_f = const.tile([1, 128], F32)
            nc.gpsimd.memset(ones_f[:], 1.0)
            ones = const.tile([1, 128], F32R)
            nc.vector.tensor_copy(ones[:], ones_f[:])
            bv1r = const.tile([1, D], F32R)
            warm = const.tile([1, 2], BF16)
            nc.scalar.activation(warm[:], ones_f[0:1, 0:2], EXP)
            # [128,128] bf16 identity for the tail's PE transposes
            id1 = const.tile([128, 128], BF16)
            nc.gpsimd.memset(id1[:], 1.0)
            ident = const.tile([128, 128], BF16)
            nc.gpsimd.affine_select(ident[:], id1[:], pattern=[[1, 128]],
                                    compare_op=mybir.AluOpType.is_equal,
                                    fill=0.0, base=0, channel_multiplier=-1)

            # ---- persistent activations ----
            qT = pers.tile([128, 2, S], BF16)   # [d_local, dh, s]
            kT = pers.tile([128, 2, S], BF16)
            vS = pers.tile([128, SKT, 4, HD + 1], BF16)  # [sk, skt, head, d|1]
            nc.gpsimd.memset(vS[:], 1.0)       # ones column (rest overwritten)
            attnT = pers.tile([128, 2, S], BF16)  # [d in pair, hp, sq]

            ps_pj = ctx.enter_context(
                tc.tile_pool(name="ps_pj", bufs=2, space="PSUM"))
            ps_qk = ctx.enter_context(
                tc.tile_pool(name="ps_qk", bufs=4, space="PSUM"))
            ps_av = ctx.enter_context(
                tc.tile_pool(name="ps_av", bufs=2, space="PSUM"))

            bvb = const.tile([128, D], F32)

            def load_xt(xd, sb, name):
                """DMA one s-block of pre-transposed x: [128h, HT, SB] bf16."""
                xt = xt_p.tile([128, HT, SB], BF16, tag="xt", name=name)
                nc.sync.dma_start(
                    xt[:], xd.rearrange("(j p) s -> p j s", p=128)[
                        :, :, ts(sb, SB)])
                return xt

            def proj_dmajor_unit(xt, w, bias2, dst, sb, dh, c0=0, c1=SB):
                # dst[:, dh, sb*SB+c0:+c1] = (x @ w + b).T (d-major)
                pp = ps_pj.tile([128, 512], F32, tag="pj", name="pp")
                for j in range(HT):
                    nc.tensor.matmul(pp[:, 0:c1 - c0], w[:, j, ts(dh, 128)],
                                     xt[:, j, c0:c1],
                                     start=(j == 0), stop=(j == HT - 1))
                nc.vector.tensor_scalar_add(
                    dst[:, dh, sb * SB + c0:sb * SB + c1], pp[:, 0:c1 - c0],
                    bias2[:, dh:dh + 1])

            def qk_score_h(hp, sqb, sk, hh):
                # one head's [128 sk, 512 sq] score tile: a single PSUM
                # bank, so the 4-deep ring gives two full tiles of exp
                # lookahead (the exp+semaphore round trip is ~1us while the
                # PE's per-tile work is ~0.65us).
                pqk = ps_qk.tile([128, 512], F32, tag="qk", name="pqk")
                r0 = HD * hh
                nc.tensor.matmul(
                    pqk[:],
                    kT[r0:r0 + HD, hp, ts(sk, 128)],
                    qT[r0:r0 + HD, hp, ts(sqb, SB)],
                    start=True, stop=True)
                return pqk

            def qk_exp_pre(hp, sqb, sk, eng="aa"):
                # pre-tile path (projection phase): per-head tiles, engine
                # per half given by `eng` (a=ACT, d=DVE).
                ets = []
                for hh in range(2):
                    pqk = qk_score_h(hp, sqb, sk, hh)
                    et = pre_p.tile([128, 512], BF16, tag="e", name="et")
                    if eng[hh] == "a":
                        nc.scalar.activation(et[:], pqk[:], EXP, scale=SCALE)
                    else:
                        dve_exp(et[:], pqk[:])
                    ets.append(et)
                return tuple(ets)

            def qk_exp_split(hp, sqb, sk):
                # inline path: head 0 -> ACT, head 1 -> DVE, concurrently.
                pqk0 = qk_score_h(hp, sqb, sk, 0)
                et0 = eh_p.tile([128, 512], BF16, tag="eh", name="et0")
                nc.scalar.activation(et0[:], pqk0[:], EXP, scale=SCALE)
                pqk1 = qk_score_h(hp, sqb, sk, 1)
                et1 = eh_p.tile([128, 512], BF16, tag="eh", name="et1")
                dve_exp(et1[:], pqk1[:])
                return (et0, et1)

            def av_accum(hp, sk, et, pav):
                # pav[hh][:, sqc, 0:65] += et[hh][:,chunk].T @ v+ones
                # start=True marks the whole 2KB PSUM bank pending-zero, so
                # only the bank's first group may use it; later groups'
                # first accumulate reads pending-zero bytes as zero.
                for hh in range(2):
                    for sqc in range(4):
                        nc.tensor.matmul(
                            pav[hh][:, sqc, 0:HD + 1],
                            et[hh][:, ts(sqc, 128)],
                            vS[:, sk, 2 * hp + hh, :],
                            start=(sk == 0 and sqc == 0),
                            stop=(sk == SKT - 1),
                            skip_group_check=True)

            def emit_outproj_mm(sqb, st):
                # po matmuls only; the ACT drain copies + y DMA are emitted
                # later (emit_outproj_drain) so they sit BEHIND the next exp
                # in the in-order ACT stream instead of blocking it.
                fin = fin_p.tile([128, H], BF16, tag="fin", name="fin")
                pos = []
                for eb in range(2):
                    po = ps_pj.tile([128, 512], F32, tag="pj", name="po")
                    nc.tensor.matmul(po[:],
                                     attnT[:, 0, ts(4 * sqb + st, 128)],
                                     wo[:, 0, ts(eb, 512)],
                                     start=True, stop=False,
                                     skip_group_check=True)
                    nc.tensor.matmul(po[:],
                                     attnT[:, 1, ts(4 * sqb + st, 128)],
                                     wo[:, 1, ts(eb, 512)],
                                     start=False, stop=True,
                                     skip_group_check=True)
                    pos.append(po)
                return (sqb, st, fin, pos)

            def emit_outproj_drain(w):
                # one PSUM->SBUF copy per engine so neither in-order queue
                # eats both
                sqb, st, fin, pos = w
                nc.scalar.activation(fin[:, ts(0, 512)], pos[0][:], COPY)
                nc.vector.tensor_copy(fin[:, ts(1, 512)], pos[1][:])
                nc.sync.dma_start(y[ts(4 * sqb + st, 128), :], fin[:])

            def attn_finish(hp, sqb, pav, fuse_outproj=False):
                # normalize out of PSUM (per-partition denominators in col 64)
                # and transpose [sq, d] -> attnT[d, sq] on the DMA XBAR (SP).
                # The per-chunk scale multiplies are split across DVE
                # (tensor_scalar) and ACT (Copy with per-partition scale) to
                # halve the batch-boundary normalize latency.
                atn = [atn_p.tile([128, 128], BF16, tag="atn", name="atn")
                       for _ in range(4)]
                recs = []
                for hh in range(2):
                    rec = small.tile([128, 4], F32, tag="rec", name="rec")
                    with nc.allow_low_precision(reason="softmax denom recip"):
                        nc.vector.reciprocal(rec[:], pav[hh][:, :, HD:HD + 1])
                    recs.append(rec)
                # tail fast-drain for the last batch: every chunk's head-0
                # outproj mm runs DURING the normalize+transpose window (its
                # attnT half landed a batch ago), on PSUM borrowed from the
                # now-idle qk/pj rings; chunk 3 takes the av banks once the
                # normalize reads drain. Chunk k then completes (head-1 mm +
                # drain + y DMA) as its transpose lands. Transposes alternate
                # SP/ACT queues so their configs don't serialize.
                tails = []
                if fuse_outproj:
                    for st in range(2):
                        fin = fin_p.tile([128, H], BF16, tag="fin",
                                         name="fin")
                        pos = []
                        for eb in range(2):
                            po = ps_qk.tile([128, 512], F32, tag="qk",
                                            name="pot")
                            nc.tensor.matmul(
                                po[:], attnT[:, 0, ts(4 * sqb + st, 128)],
                                wo[:, 0, ts(eb, 512)], start=True,
                                stop=False, skip_group_check=True)
                            pos.append(po)
                        tails.append((st, fin, pos))
                for sqc in range(4):
                    nc.vector.tensor_scalar_mul(
                        atn[sqc][:, ts(0, HD)],
                        pav[0][:, sqc, 0:HD],
                        recs[0][:, sqc:sqc + 1])
                    nc.scalar.activation(
                        atn[sqc][:, ts(1, HD)],
                        pav[1][:, sqc, 0:HD], COPY,
                        scale=recs[1][:, sqc:sqc + 1])
                    if not fuse_outproj:
                        nc.sync.dma_start_transpose(
                            attnT[:, hp, ts(4 * sqb + sqc, 128)],
                            atn[sqc][:])
                    else:
                        # tail: transpose on the PE (53ns + a short copy)
                        # instead of the ~2.3us DMA XBAR round trip; copies
                        # alternate DVE/ACT.
                        ptr = ps_pj.tile([128, 128], BF16, tag="pj",
                                         name="ptr")
                        nc.tensor.matmul(ptr[:], atn[sqc][:], ident[:],
                                         is_transpose=True,
                                         skip_group_check=True)
                        dst = attnT[:, hp, ts(4 * sqb + sqc, 128)]
                        if sqc % 2:
                            nc.scalar.activation(dst, ptr[:], COPY)
                        else:
                            nc.vector.tensor_copy(dst, ptr[:])
                if fuse_outproj:
                    # chunk 3's head-0 mms go to the av banks -- emitted
                    # after the muls above so the WAR on the freshly-read
                    # pav banks is tracked; chunk 2 reuses the pj ring after
                    # the transpose copies drain it.
                    for st in (3, 2):
                        fin = fin_p.tile([128, H], BF16, tag="fin",
                                         name="fin")
                        pos = []
                        for eb in range(2):
                            if st == 3:
                                pot = ps_av.tile([128, 4, 128], F32,
                                                 tag="av", name="pot")
                                po = pot.rearrange("p a b -> p (a b)")
                            else:
                                po = ps_pj.tile([128, 512], F32, tag="pj",
                                                name="pot")
                            nc.tensor.matmul(
                                po[:], attnT[:, 0, ts(4 * sqb + st, 128)],
                                wo[:, 0, ts(eb, 512)], start=True,
                                stop=False, skip_group_check=True)
                            pos.append(po)
                        tails.append((st, fin, pos))
                    tails.sort()
                    for st, fin, pos in tails:
                        for eb in range(2):
                            nc.tensor.matmul(
                                pos[eb][:],
                                attnT[:, 1, ts(4 * sqb + st, 128)],
                                wo[:, 1, ts(eb, 512)], start=False,
                                stop=True, skip_group_check=True)
                        nc.scalar.activation(fin[:, ts(0, 512)], pos[0][:],
                                             COPY)
                        nc.vector.tensor_copy(fin[:, ts(1, 512)], pos[1][:])
                        nc.sync.dma_start(y[ts(4 * sqb + st, 128), :],
                                          fin[:])

            # ---- streaming loads + projections, with scores+exp for ready
            # (sqb, hp, sk) tiles pre-emitted in consumption order so both
            # exp engines start chewing softmax work early. ----
            def spread(p):
                return sorted({int(round(i * SKT / p)) for i in range(p)})

            # pre-tiles: ACT-only (a DVE pre-exp would head-of-line-block
            # the projection epilogue adds on the in-order DVE queue and
            # stall the PE's ps_pj ring). Spread within each batch so the
            # inline ACT/DVE load stays even through the attention stream.
            # leading + trailing sks per batch: a pre-covered batch END lets
            # the engine queues drain before the normalize, so the boundary
            # WAR on the pav ring resolves fast; a pre-covered batch START
            # gives the PE immediate AV work after it. b00 (consumed inside
            # the projection phase) stays ACT-only; later pre tiles put one
            # half on DVE -- at most one 0.66us DVE op lands between
            # projection epilogue adds, within the pp ring's 2-unit slack.
            PRE_SPEC = [((0, 0), list(range(SKT)), "aa"),
                        ((0, 1), [0, 1, 13, 14, 15], "ad"),
                        ((1, 0), [0, 12, 13, 14, 15], "ad"),
                        ((1, 1), [0, 12, 13, 14, 15], "ad"),
                        ((2, 0), [0, 13, 14, 15], "ad"),
                        ((2, 1), [0, 13, 14, 15], "ad"),
                        ((3, 0), [0, 14, 15], "ad")]
            pre_order = []
            for (sqb, hp), sks, eng in PRE_SPEC:
                for sk in sks:
                    pre_order.append((sqb, hp, sk, eng))
            pre = {}
            st_pre = {"i": 0, "q": set(), "k": set()}

            def emit_pre(limit):
                # dh-granular readiness: head-pair hp only needs the dh=hp
                # halves of its qT/kT blocks.
                done = 0
                while st_pre["i"] < len(pre_order) and done < limit:
                    sqb, hp, sk, eng = pre_order[st_pre["i"]]
                    if (sqb, hp) not in st_pre["q"] or \
                            (sk // 4, hp) not in st_pre["k"]:
                        break
                    pre[(sqb, hp, sk)] = qk_exp_pre(hp, sqb, sk, eng)
                    st_pre["i"] += 1
                    done += 1

            def proj_qk(xd, w, bias2, dst, sb, which):
                xt = load_xt(xd, sb, "xt" + which)
                for dh in range(2):
                    proj_dmajor_unit(xt, w, bias2, dst, sb, dh)
                    st_pre[which].add((sb, dh))
                    emit_pre(3)

            # batch (0,0)'s AV interleaves into the V phase: its et tiles
            # are all precomputed, and vS[sk] is ready right after block
            # sk//4's epilogue -- so the first attention batch costs no
            # wall-clock of its own.
            pav00 = [ps_av.tile([128, 4, 128], F32, tag="av", name=f"pav{hh}")
                     for hh in range(2)]

            def proj_v(sb):
                # batch (0,0)'s AV trails the V epilogues by one si unit so
                # the in-order PE never waits on the DVE vS write latency.
                xtv = load_xt(xv, sb, "xtv")
                for si in range(4):
                    pv = ps_pj.tile([128, 512], F32, tag="pj", name="pv")
                    for j in range(HT):
                        nc.tensor.matmul(pv[:, 0:D],
                                         xtv[:, j, ts(si, 128)],
                                         wv[:, j, :],
                                         start=(j == 0), stop=(j == HT - 1))
                    nc.vector.tensor_add(
                        vS[:, 4 * sb + si, :, 0:HD],
                        pv[:, 0:D].rearrange("p (g d) -> p g d", g=4),
                        bvb[:].rearrange("p (g d) -> p g d", g=4))
                    emit_pre(3)
                    sk = 4 * sb + si
                    if sk > 0:
                        av_accum(0, sk - 1, pre.pop((0, 0, sk - 1)), pav00)

            # Q0 then all K (unlocks every sqb0 tile), then Q1-3 (unlocks
            # the rest), V last (first consumed only once attention starts).
            # xtq0 lands in two halves so the first projection matmul can
            # start on the first 256 columns while the rest streams in.
            xtq0 = xt_p.tile([128, HT, SB], BF16, tag="xt", name="xtq0")
            xq_r = xq.rearrange("(j p) s -> p j s", p=128)
            nc.sync.dma_start(xtq0[:, :, 0:256], xq_r[:, :, 0:256])
            nc.sync.dma_start(xtq0[:, :, 256:512], xq_r[:, :, 256:512])
            nc.scalar.dma_start(bq2[:], bq_d[:])
            wk_r = wk_d.rearrange("(j p) d -> p j d", p=128)
            nc.scalar.dma_start(wk[:, :, 0:128], wk_r[:, :, 0:128])
            # fast start: narrow first xk load + mini K projection puts the
            # first score+exp on the engines early.
            xtk0a = xt_p.tile([128, HT, 128], BF16, tag="xta", name="xtk0a", bufs=1)
            nc.sync.dma_start(wq[:, :, 128:256], wq_r[:, :, 128:256])
            nc.sync.dma_start(
                xtk0a[:], xk.rearrange("(j p) s -> p j s", p=128)[:, :, 0:128])
            nc.scalar.dma_start(bk2[:], bk_d[:])
            nc.scalar.dma_start(wk[:, :, 128:256], wk_r[:, :, 128:256])
            proj_dmajor_unit(xtq0, wq, bq2, qT, 0, 0, 0, 256)
            proj_dmajor_unit(xtq0, wq, bq2, qT, 0, 0, 256, 512)
            st_pre["q"].add((0, 0))
            pk0 = ps_pj.tile([128, 512], F32, tag="pj", name="pk0")
            for j in range(HT):
                nc.tensor.matmul(pk0[:, 0:128], wk[:, j, 0:128],
                                 xtk0a[:, j, :],
                                 start=(j == 0), stop=(j == HT - 1))
            nc.vector.tensor_scalar_add(kT[:, 0, 0:128], pk0[:, 0:128],
                                        bk2[:, 0:1])
            pre[(0, 0, 0)] = qk_exp_pre(0, 0, 0, "aa")
            proj_dmajor_unit(xtq0, wq, bq2, qT, 0, 1)
            st_pre["q"].add((0, 1))
            st_pre["i"] = 1
            xtk0 = load_xt(xk, 0, "xtk")
            proj_dmajor_unit(xtk0, wk, bk2, kT, 0, 0, 128, SB)
            st_pre["k"].add((0, 0))
            emit_pre(3)
            proj_dmajor_unit(xtk0, wk, bk2, kT, 0, 1)
            st_pre["k"].add((0, 1))
            emit_pre(3)
            for sb in range(1, NSB):
                proj_qk(xk, wk, bk2, kT, sb, "k")
                emit_pre(3)
            for sb in range(1, NSB):
                proj_qk(xq, wq, bq2, qT, sb, "q")
                emit_pre(3)
            nc.sync.dma_start(wv[:], wv_d.rearrange("(j p) d -> p j d", p=128))
            # v-bias broadcast, deferred here so its small DMAs stay off the
            # critical startup path (first needed by V0's epilogue)
            nc.sync.dma_start(bv1[:], bv_d[:])
            nc.vector.tensor_copy(bv1r[:], bv1[:])
            pbc = ps_pj.tile([128, 512], F32, tag="pj", name="pbc")
            nc.tensor.matmul(pbc[:, 0:D], ones[0:1, :], bv1r[:])
            nc.vector.tensor_copy(bvb[:], pbc[:, 0:D])
            for sb in range(NSB):
                proj_v(sb)
            av_accum(0, SKT - 1, pre.pop((0, 0, SKT - 1)), pav00)
            emit_pre(len(pre_order))

            # deferred: output-projection weights (first needed ~60us in)
            wo = const.tile([128, 2, H], BF16)
            nc.sync.dma_start(wo[:], wo_d.rearrange("(i p) e -> p i e", p=128))

            # ---- attention stream. Inline tiles split each head pair's
            # exp across ACT/DVE (separate PSUM banks), with 2-item score
            # lookahead across batch boundaries. outproj(sqb-1) chunks are
            # spread through the following batch (sk = 2,5,8,11) so the
            # ps_pj ring never backs up on the ACT drain. ----
            attn_finish(0, 0, pav00)
            batches = [(s, h) for s in range(NSB) for h in range(2)][1:]
            stream = [(s, h, k) for (s, h) in batches for k in range(SKT)]
            emitted = {}
            st_la = {"ep": 0}

            def pump(upto, inline_ahead=0):
                # emit score+exp for stream items <= upto, plus keep
                # `inline_ahead` INLINE (non-pre) tiles in flight beyond
                # the consumer. `emitted` holds exactly the un-consumed
                # inline tiles, so len(emitted) IS the in-flight count --
                # pre-covered items don't eat the lookahead window.
                while st_la["ep"] < len(stream) and \
                        (st_la["ep"] <= upto or len(emitted) < inline_ahead):
                    key = stream[st_la["ep"]]
                    if key not in pre:
                        emitted[key] = qk_exp_split(key[1], key[0], key[2])
                    st_la["ep"] += 1

            deferred = []
            pending_drain = []
            pav_cur = {}
            for ci, key in enumerate(stream):
                sqb, hp, sk = key
                if sk == 0:
                    pav_cur[(sqb, hp)] = [
                        ps_av.tile([128, 4, 128], F32, tag="av",
                                   name=f"pav{hh}") for hh in range(2)]
                # ensure the current tile's exp exists; the lookahead pump
                # at loop end runs AFTER any finish so normalize ops aren't
                # queued behind the next batch's exps.
                pump(ci)
                if pending_drain:
                    emit_outproj_drain(pending_drain.pop(0))
                et = pre.pop(key, None)
                if et is None:
                    et = emitted.pop(key)
                av_accum(hp, sk, et, pav_cur[(sqb, hp)])
                if deferred and sk in (2, 5, 8, 11):
                    dq, dst = deferred[0]
                    pending_drain.append(emit_outproj_mm(dq, dst))
                    if dst == 3:
                        deferred.pop(0)
                    else:
                        deferred[0] = (dq, dst + 1)
                if sk == SKT - 1:
                    last = (sqb == NSB - 1 and hp == 1)
                    attn_finish(hp, sqb, pav_cur.pop((sqb, hp)),
                                fuse_outproj=last)
                    if hp == 0 and sqb > 0:
                        deferred.append((sqb - 1, 0))
                pump(ci, inline_ahead=2)

    nc.compile()
    return nc


def _get_nc():
    if "nc" not in _CACHE:
        _CACHE["nc"] = _build()
    return _CACHE["nc"]


def _kernel_numpy(query, key, value, attention_mask,
                  Wq, bq, Wk, bk, Wv, bv, Wo, bo):
    """Exact fp32 numpy fallback (only used for inputs outside the spec:
    nonzero mask or unexpected shapes)."""
    B, S_, H_ = query.shape
    NH = 16
    HDl = H_ // NH
    q = query @ Wq + bq
    k = key @ Wk + bk
    v = value @ Wv + bv

    def split(x):
        return x.reshape(B, S_, NH, HDl).transpose(0, 2, 1, 3)

    q, k, v = split(q), split(k), split(v)
    s = np.einsum("bhqd,bhkd->bhqk", q, k) / np.sqrt(np.float32(HDl))
    s = s + attention_mask[:, None, :, :]
    s = s - s.max(axis=-1, keepdims=True)
    e = np.exp(s)
    w = e / e.sum(axis=-1, keepdims=True)
    o = np.einsum("bhqk,bhkd->bhqd", w, v)
    o = o.transpose(0, 2, 1, 3).reshape(B, S_, H_)
    return (o @ Wo + bo).astype(np.float32)


def kernel(query, key, value, attention_mask, Wq, bq, Wk, bk, Wv, bv, Wo, bo):
    query = np.asarray(query, np.float32)
    key = np.asarray(key, np.float32)
    value = np.asarray(value, np.float32)
    Wq, Wk, Wv, Wo = (np.asarray(a, np.float32) for a in (Wq, Wk, Wv, Wo))
    bq, bk, bv, bo = (np.asarray(a, np.float32) for a in (bq, bk, bv, bo))
    attention_mask = np.asarray(attention_mask, np.float32)

    if query.shape != (2, S, H) or Wq.shape != (H, H) or \
            attention_mask.shape != (2, S, S) or np.any(attention_mask):
        return _kernel_numpy(query, key, value, attention_mask,
                             Wq, bq, Wk, bk, Wv, bv, Wo, bo)

    qT = [np.ascontiguousarray(query[b].astype(ml_dtypes.bfloat16).T)
          for b in range(2)]
    kTh = [np.ascontiguousarray(key[b].astype(ml_dtypes.bfloat16).T)
           for b in range(2)]
    vTh = [np.ascontiguousarray(value[b].astype(ml_dtypes.bfloat16).T)
           for b in range(2)]

    nc = _get_nc()
    in_maps = []
    for c in range(N_CORES):
        b, hg = divmod(c, 4)
        sl = slice(D * hg, D * hg + D)
        in_maps.append({
            "xqT": qT[b],
            "xkT": kTh[b],
            "xvT": vTh[b],
            "wq": np.ascontiguousarray(Wq[:, sl]).astype(ml_dtypes.bfloat16),
            "wk": np.ascontiguousarray(Wk[:, sl]).astype(ml_dtypes.bfloat16),
            "wv": np.ascontiguousarray(Wv[:, sl]).astype(ml_dtypes.bfloat16),
            "wo": np.ascontiguousarray(Wo[sl, :]).astype(ml_dtypes.bfloat16),
            "bq2": bq[sl].reshape(2, 128).T.copy(),
            "bk2": bk[sl].reshape(2, 128).T.copy(),
            "bv1": bv[sl].reshape(1, D).copy(),
        })
    try:
        res = run_bass_kernel_spmd(nc, in_maps, list(range(N_CORES)))
    finally:
        # run_bass_via_pjrt monkeypatches libneuronxla.neuronx_cc; restore it
        # so later ordinary jax compiles in the caller's process are untouched.
        try:
            import libneuronxla  # pyright: ignore[reportMissingImports]
            if hasattr(libneuronxla, "orig_neuronx_cc"):
                libneuronxla.neuronx_cc = libneuronxla.orig_neuronx_cc
        except ImportError:
            pass
    outs = [res.results[c]["y"] for c in range(N_CORES)]
    out = np.empty((2, S, H), np.float32)
    for b in range(2):
        out[b] = (outs[4 * b].astype(np.float32)
                  + outs[4 * b + 1].astype(np.float32)
                  + outs[4 * b + 2].astype(np.float32)
                  + outs[4 * b + 3].astype(np.float32)) + bo
    return out


# revision 32
# speedup vs baseline: 1.0260x; 1.0057x over previous
"""Multi-head attention (B=2, S=2048, H=1024, NH=16) on 8 TRN2 NeuronCores.

Sharding: core c -> (batch b = c//4, head-group hg = c%4). Each core computes
Q/K/V projections for its 4 heads (256 columns of Wq/Wk/Wv), attention for
those heads, and a partial output projection (its 256 rows of Wo). Host sums
the 4 partials per batch and adds bo.

Per-core device pipeline (all matmuls at 1 cycle/row via bf16):
  - x is pre-transposed + bf16-cast on the host, so h-major xT streams
    straight into the projections.
  - Q/K projections produce qT/kT d-major [256, 2048] (W stationary);
    V s-major [2048, 4, 65] bf16 (xT stationary) with a ones column appended.
  - scoresT[sk, sq] per head pair: lhsT = kT head slice (K=64); the two
    heads land in the two PSUM banks of one [128, 2, 512] tile.
  - softmax exp runs on BOTH ACT and DVE concurrently: ACT exps head 0's
    bank (table exp, scale=1/8 fused); DVE exps head 1's bank with a custom
    microcoded op (deg-3 Horner + 2 squarings ~ exp(x/8), max rel err 1.7%
    at the +-3.6-sigma tails, 0.4% in the bulk -- end-to-end rel err 0.006
    vs 2e-2 budget). This doubles exp bandwidth so the PE never starves in
    the attention phase; pre-computed full tiles during the projection
    phase (on both engines) cover the remaining shortfall.
  - AV in [sq, d] orientation: lhsT = et 128-col chunk, rhs = v+ones
    [128, 65] -> psum [sq 128, 65]; col 64 accumulates the softmax
    denominators for free.
  - Normalization: batched DVE reciprocal + TensorScalarPtr multiply per
    (head, sq-chunk) writes normalized attn [sq, d] bf16 out of PSUM.
  - attn -> attnT via DMA XBAR transpose on the SP queue.
  - Output projection all-bf16: attnT stationary, Wo rows moving; PSUM
    drained by ACT copies (no bias add on device -- host adds bo), y DMA'd
    out per 128-row chunk, chunks spread across the following batch.
PSUM budget (8 banks): 2 proj/outproj + 2x2 score double-buffer + 2 AV.
"""
import os
import sys

if os.path.isdir("/opt/trn_rl_repo"):
    sys.path.insert(0, "/opt/trn_rl_repo")

from contextlib import ExitStack

import numpy as np
import ml_dtypes

import concourse.bass as bass
import concourse.tile as tile
from concourse import bacc, mybir
from concourse.bass import ts
from concourse.bass_utils import run_bass_kernel_spmd

F32 = mybir.dt.float32
F32R = mybir.dt.float32r
BF16 = mybir.dt.bfloat16
EXP = mybir.ActivationFunctionType.Exp
COPY = mybir.ActivationFunctionType.Copy

S = 2048
H = 1024
D = 256          # per-core head-slice width (4 heads x 64)
HD = 64
N_CORES = 8
SB = 512         # s-block
NSB = S // SB    # 4
HT = H // 128    # 8 h-tiles
SKT = S // 128   # 16 sk-tiles
SCALE = 1.0 / 8.0  # 1/sqrt(HD)

# deg-3 Horner coefficients for the DVE exp op, raw-score basis:
# exp(u/8) ~ (((1 + u*(EC0 + u*(EC1 + u*EC2)))^2)^2, |u/8| <= 3.6
EC0 = 0.031503140926361084
EC1 = 0.0005148400668986142
EC2 = 4.80940570923849e-06

_CACHE = {}


def _register_exp_op():
    """Define + register the custom DVE op EXP_P3SQSQ_ANT (idempotent).
    Registration appends to dve_ops.OPS at runtime so the per-NEFF DVE
    table generation and row assignment pick it up without editing the
    repo."""
    if "exp_op" in _CACHE:
        return _CACHE["exp_op"]
    import concourse.dve_ops as dvo
    from concourse.dve_spec import Spec, Src0, C0, C1, C2, One, sq, lower, \
        _has_src1
    from concourse.dve_uop import DveOpSpec

    name = "EXP_P3SQSQ_ANT"
    for o in dvo.OPS:
        if o.name == name:
            _CACHE["exp_op"] = o
            return o
    body = sq(sq(One + Src0 * (C0 + Src0 * (C1 + Src0 * C2))))

    def ref(in0, in1, c0, c1, c2):
        p = (1.0 + in0 * (c0 + in0 * (c1 + in0 * c2))).astype(np.float32)
        p = (p * p).astype(np.float32)
        return (p * p).astype(np.float32)

    spec = Spec(body=body, reference=ref)
    row = dvo._CUSTOM_DVE_ROW_BASE + len(dvo.OPS)
    assert row < 0x20
    shas = {}
    for ver in ("v3", "v4"):
        try:
            uops = lower(spec, ver=ver)
            shas[ver] = DveOpSpec(name=name, opcode=row, uops=uops,
                                  rd1_en=_has_src1(spec)).sha(ver)
        except Exception:
            if ver == "v3":
                raise
    op = dvo.DveOp(name, spec, subdim=False, uops_sha=shas)
    dvo.OPS.append(op)
    dvo._SUB_OPCODE_FOR_NAME[name] = row
    dvo.CUSTOM_DVE_SPECS[name] = spec
    _CACHE["exp_op"] = op
    return op


def _build():
    exp_op = _register_exp_op()
    nc = bacc.Bacc("TRN2", target_bir_lowering=False, debug=False,
                   num_devices=N_CORES)

    xq = nc.dram_tensor("xqT", [H, S], BF16, kind="ExternalInput").ap()
    xk = nc.dram_tensor("xkT", [H, S], BF16, kind="ExternalInput").ap()
    xv = nc.dram_tensor("xvT", [H, S], BF16, kind="ExternalInput").ap()
    wq_d = nc.dram_tensor("wq", [H, D], BF16, kind="ExternalInput").ap()
    wk_d = nc.dram_tensor("wk", [H, D], BF16, kind="ExternalInput").ap()
    wv_d = nc.dram_tensor("wv", [H, D], BF16, kind="ExternalInput").ap()
    wo_d = nc.dram_tensor("wo", [D, H], BF16, kind="ExternalInput").ap()
    bq_d = nc.dram_tensor("bq2", [128, 2], F32, kind="ExternalInput").ap()
    bk_d = nc.dram_tensor("bk2", [128, 2], F32, kind="ExternalInput").ap()
    bv_d = nc.dram_tensor("bv1", [1, D], F32, kind="ExternalInput").ap()
    y = nc.dram_tensor("y", [S, H], BF16, kind="ExternalOutput").ap()

    def dve_exp(out_ap, in_ap):
        nc.vector._custom_dve(exp_op, out=out_ap, in0=in_ap,
                              s0=EC0, s1=EC1, imm2=EC2)

    with tile.TileContext(nc) as tc:
        with ExitStack() as ctx:
            const = ctx.enter_context(tc.tile_pool(name="const", bufs=1))
            pers = ctx.enter_context(tc.tile_pool(name="pers", bufs=1))
            xt_p = ctx.enter_context(tc.tile_pool(name="xt", bufs=2))
            small = ctx.enter_context(tc.tile_pool(name="small", bufs=4))
            pre_p = ctx.enter_context(tc.tile_pool(name="prep", bufs=94))
            eh_p = ctx.enter_context(tc.tile_pool(name="ehp", bufs=10))
            atn_p = ctx.enter_context(tc.tile_pool(name="atnp", bufs=6))
            fin_p = ctx.enter_context(tc.tile_pool(name="finp", bufs=4))

            # ---- constants ----
            # weights + biases go out on the ACT HWDGE queue so their
            # descriptor generation runs in parallel with the SP queue's
            # xt streams (two HWDGEs).
            wq = const.tile([128, HT, D], BF16)
            wq_r = wq_d.rearrange("(j p) d -> p j d", p=128)
            nc.scalar.dma_start(wq[:, :, 0:128], wq_r[:, :, 0:128])
            wk = const.tile([128, HT, D], BF16)
            wv = const.tile([128, HT, D], BF16)
            bq2 = const.tile([128, 2], F32)
            bk2 = const.tile([128, 2], F32)
            bv1 = const.tile([1, D], F32)
            ones_f = const.tile([1, 128], F32)
            nc.gpsimd.memset(ones_f[:], 1.0)
            ones = const.tile([1, 128], F32R)
            nc.vector.tensor_copy(ones[:], ones_f[:])
            bv1r = const.tile([1, D], F32R)
            warm = const.tile([1, 2], BF16)
            nc.scalar.activation(warm[:], ones_f[0:1, 0:2], EXP)
            # [128,128] bf16 identity for the tail's PE transposes
            id1 = const.tile([128, 128], BF16)
            nc.gpsimd.memset(id1[:], 1.0)
            ident = const.tile([128, 128], BF16)
            nc.gpsimd.affine_select(ident[:], id1[:], pattern=[[1, 128]],
                                    compare_op=mybir.AluOpType.is_equal,
                                    fill=0.0, base=0, channel_multiplier=-1)

            # ---- persistent activations ----
            qT = pers.tile([128, 2, S], BF16)   # [d_local, dh, s]
            kT = pers.tile([128, 2, S], BF16)
            vS = pers.tile([128, SKT, 4, HD + 1], BF16)  # [sk, skt, head, d|1]
            nc.gpsimd.memset(vS[:], 1.0)       # ones column (rest overwritten)
            attnT = pers.tile([128, 2, S], BF16)  # [d in pair, hp, sq]

            ps_pj = ctx.enter_context(
                tc.tile_pool(name="ps_pj", bufs=2, space="PSUM"))
            ps_qk = ctx.enter_context(
                tc.tile_pool(name="ps_qk", bufs=4, space="PSUM"))
            ps_av = ctx.enter_context(
                tc.tile_pool(name="ps_av", bufs=2, space="PSUM"))

            bvb = const.tile([128, D], F32)

            def load_xt(xd, sb, name):
                """DMA one s-block of pre-transposed x: [128h, HT, SB] bf16."""
                xt = xt_p.tile([128, HT, SB], BF16, tag="xt", name=name)
                nc.sync.dma_start(
                    xt[:], xd.rearrange("(j p) s -> p j s", p=128)[
                        :, :, ts(sb, SB)])
                return xt

            def proj_dmajor_unit(xt, w, bias2, dst, sb, dh, c0=0, c1=SB):
                # dst[:, dh, sb*SB+c0:+c1] = (x @ w + b).T (d-major)
                pp = ps_pj.tile([128, 512], F32, tag="pj", name="pp")
                for j in range(HT):
                    nc.tensor.matmul(pp[:, 0:c1 - c0], w[:, j, ts(dh, 128)],
                                     xt[:, j, c0:c1],
                                     start=(j == 0), stop=(j == HT - 1))
                nc.vector.tensor_scalar_add(
                    dst[:, dh, sb * SB + c0:sb * SB + c1], pp[:, 0:c1 - c0],
                    bias2[:, dh:dh + 1])

            def qk_score_h(hp, sqb, sk, hh):
                # one head's [128 sk, 512 sq] score tile: a single PSUM
                # bank, so the 4-deep ring gives two full tiles of exp
                # lookahead (the exp+semaphore round trip is ~1us while the
                # PE's per-tile work is ~0.65us).
                pqk = ps_qk.tile([128, 512], F32, tag="qk", name="pqk")
                r0 = HD * hh
                nc.tensor.matmul(
                    pqk[:],
                    kT[r0:r0 + HD, hp, ts(sk, 128)],
                    qT[r0:r0 + HD, hp, ts(sqb, SB)],
                    start=True, stop=True)
                return pqk

            def qk_exp_pre(hp, sqb, sk, eng="aa"):
                # pre-tile path (projection phase): per-head tiles, engine
                # per half given by `eng` (a=ACT, d=DVE).
                ets = []
                for hh in range(2):
                    pqk = qk_score_h(hp, sqb, sk, hh)
                    et = pre_p.tile([128, 512], BF16, tag="e", name="et")
                    if eng[hh] == "a":
                        nc.scalar.activation(et[:], pqk[:], EXP, scale=SCALE)
                    else:
                        dve_exp(et[:], pqk[:])
                    ets.append(et)
                return tuple(ets)

            def qk_exp_split(hp, sqb, sk):
                # inline path: head 0 -> ACT, head 1 -> DVE, concurrently.
                pqk0 = qk_score_h(hp, sqb, sk, 0)
                et0 = eh_p.tile([128, 512], BF16, tag="eh", name="et0")
                nc.scalar.activation(et0[:], pqk0[:], EXP, scale=SCALE)
                pqk1 = qk_score_h(hp, sqb, sk, 1)
                et1 = eh_p.tile([128, 512], BF16, tag="eh", name="et1")
                dve_exp(et1[:], pqk1[:])
                return (et0, et1)

            def av_accum(hp, sk, et, pav):
                # pav[hh][:, sqc, 0:65] += et[hh][:,chunk].T @ v+ones
                # start=True marks the whole 2KB PSUM bank pending-zero, so
                # only the bank's first group may use it; later groups'
                # first accumulate reads pending-zero bytes as zero.
                for hh in range(2):
                    for sqc in range(4):
                        nc.tensor.matmul(
                            pav[hh][:, sqc, 0:HD + 1],
                            et[hh][:, ts(sqc, 128)],
                            vS[:, sk, 2 * hp + hh, :],
                            start=(sk == 0 and sqc == 0),
                            stop=(sk == SKT - 1),
                            skip_group_check=True)

            def emit_outproj_mm(sqb, st):
                # po matmuls only; the ACT drain copies + y DMA are emitted
                # later (emit_outproj_drain) so they sit BEHIND the next exp
                # in the in-order ACT stream instead of blocking it.
                fin = fin_p.tile([128, H], BF16, tag="fin", name="fin")
                pos = []
                for eb in range(2):
                    po = ps_pj.tile([128, 512], F32, tag="pj", name="po")
                    nc.tensor.matmul(po[:],
                                     attnT[:, 0, ts(4 * sqb + st, 128)],
                                     wo[:, 0, ts(eb, 512)],
                                     start=True, stop=False,
                                     skip_group_check=True)
                    nc.tensor.matmul(po[:],
                                     attnT[:, 1, ts(4 * sqb + st, 128)],
                                     wo[:, 1, ts(eb, 512)],
                                     start=False, stop=True,
                                     skip_group_check=True)
                    pos.append(po)
                return (sqb, st, fin, pos)

            def emit_outproj_drain(w):
                # one PSUM->SBUF copy per engine so neither in-order queue
                # eats both
                sqb, st, fin, pos = w
                nc.scalar.activation(fin[:, ts(0, 512)], pos[0][:], COPY)
                nc.vector.tensor_copy(fin[:, ts(1, 512)], pos[1][:])
                nc.sync.dma_start(y[ts(4 * sqb + st, 128), :], fin[:])

            def attn_finish(hp, sqb, pav, fuse_outproj=False):
                # normalize out of PSUM (per-partition denominators in col 64)
                # and transpose [sq, d] -> attnT[d, sq] on the DMA XBAR (SP).
                # The per-chunk scale multiplies are split across DVE
                # (tensor_scalar) and ACT (Copy with per-partition scale) to
                # halve the batch-boundary normalize latency.
                atn = [atn_p.tile([128, 128], BF16, tag="atn", name="atn")
                       for _ in range(4)]
                recs = []
                for hh in range(2):
                    rec = small.tile([128, 4], F32, tag="rec", name="rec")
                    with nc.allow_low_precision(reason="softmax denom recip"):
                        nc.vector.reciprocal(rec[:], pav[hh][:, :, HD:HD + 1])
                    recs.append(rec)
                # tail fast-drain for the last batch: every chunk's head-0
                # outproj mm runs DURING the normalize+transpose window (its
                # attnT half landed a batch ago), on PSUM borrowed from the
                # now-idle qk/pj rings; chunk 3 takes the av banks once the
                # normalize reads drain. Chunk k then completes (head-1 mm +
                # drain + y DMA) as its transpose lands. Transposes alternate
                # SP/ACT queues so their configs don't serialize.
                tails = []
                if fuse_outproj:
                    for st in range(2):
                        fin = fin_p.tile([128, H], BF16, tag="fin",
                                         name="fin")
                        pos = []
                        for eb in range(2):
                            po = ps_qk.tile([128, 512], F32, tag="qk",
                                            name="pot")
                            nc.tensor.matmul(
                                po[:], attnT[:, 0, ts(4 * sqb + st, 128)],
                                wo[:, 0, ts(eb, 512)], start=True,
                                stop=False, skip_group_check=True)
                            pos.append(po)
                        tails.append((st, fin, pos))
                for sqc in range(4):
                    nc.vector.tensor_scalar_mul(
                        atn[sqc][:, ts(0, HD)],
                        pav[0][:, sqc, 0:HD],
                        recs[0][:, sqc:sqc + 1])
                    nc.scalar.activation(
                        atn[sqc][:, ts(1, HD)],
                        pav[1][:, sqc, 0:HD], COPY,
                        scale=recs[1][:, sqc:sqc + 1])
                    if not fuse_outproj:
                        nc.sync.dma_start_transpose(
                            attnT[:, hp, ts(4 * sqb + sqc, 128)],
                            atn[sqc][:])
                    else:
                        # tail: transpose on the PE (53ns + a short copy)
                        # instead of the ~2.3us DMA XBAR round trip; copies
                        # alternate DVE/ACT.
                        ptr = ps_pj.tile([128, 128], BF16, tag="pj",
                                         name="ptr")
                        nc.tensor.matmul(ptr[:], atn[sqc][:], ident[:],
                                         is_transpose=True,
                                         skip_group_check=True)
                        dst = attnT[:, hp, ts(4 * sqb + sqc, 128)]
                        if sqc % 2:
                            nc.scalar.activation(dst, ptr[:], COPY)
                        else:
                            nc.vector.tensor_copy(dst, ptr[:])
                if fuse_outproj:
                    # chunk 3's head-0 mms go to the av banks -- emitted
                    # after the muls above so the WAR on the freshly-read
                    # pav banks is tracked; chunk 2 reuses the pj ring after
                    # the transpose copies drain it.
                    for st in (3, 2):
                        fin = fin_p.tile([128, H], BF16, tag="fin",
                                         name="fin")
                        pos = []
                        for eb in range(2):
                            if st == 3:
                                pot = ps_av.tile([128, 4, 128], F32,
                                                 tag="av", name="pot")
                                po = pot.rearrange("p a b -> p (a b)")
                            else:
                                po = ps_pj.tile([128, 512], F32, tag="pj",
                                                name="pot")
                            nc.tensor.matmul(
                                po[:], attnT[:, 0, ts(4 * sqb + st, 128)],
                                wo[:, 0, ts(eb, 512)], start=True,
                                stop=False, skip_group_check=True)
                            pos.append(po)
                        tails.append((st, fin, pos))
                    tails.sort()
                    for st, fin, pos in tails:
                        for eb in range(2):
                            nc.tensor.matmul(
                                pos[eb][:],
                                attnT[:, 1, ts(4 * sqb + st, 128)],
                                wo[:, 1, ts(eb, 512)], start=False,
                                stop=True, skip_group_check=True)
                        nc.scalar.activation(fin[:, ts(0, 512)], pos[0][:],
                                             COPY)
                        nc.vector.tensor_copy(fin[:, ts(1, 512)], pos[1][:])
                        nc.sync.dma_start(y[ts(4 * sqb + st, 128), :],
                                          fin[:])

            # ---- streaming loads + projections, with scores+exp for ready
            # (sqb, hp, sk) tiles pre-emitted in consumption order so both
            # exp engines start chewing softmax work early. ----
            def spread(p):
                return sorted({int(round(i * SKT / p)) for i in range(p)})

            # pre-tiles: ACT-only (a DVE pre-exp would head-of-line-block
            # the projection epilogue adds on the in-order DVE queue and
            # stall the PE's ps_pj ring). Spread within each batch so the
            # inline ACT/DVE load stays even through the attention stream.
            # leading + trailing sks per batch: a pre-covered batch END lets
            # the engine queues drain before the normalize, so the boundary
            # WAR on the pav ring resolves fast; a pre-covered batch START
            # gives the PE immediate AV work after it. b00 (consumed inside
            # the projection phase) stays ACT-only; later pre tiles put one
            # half on DVE -- at most one 0.66us DVE op lands between
            # projection epilogue adds, within the pp ring's 2-unit slack.
            PRE_SPEC = [((0, 0), list(range(SKT)), "aa"),
                        ((0, 1), [0, 1, 13, 14, 15], "ad"),
                        ((1, 0), [0, 12, 13, 14, 15], "ad"),
                        ((1, 1), [0, 12, 13, 14, 15], "ad"),
                        ((2, 0), [0, 13, 14, 15], "ad"),
                        ((2, 1), [0, 13, 14, 15], "ad"),
                        ((3, 0), [0, 14, 15], "ad")]
            pre_order = []
            for (sqb, hp), sks, eng in PRE_SPEC:
                for sk in sks:
                    pre_order.append((sqb, hp, sk, eng))
            pre = {}
            st_pre = {"i": 0, "q": set(), "k": set()}

            def emit_pre(limit):
                # dh-granular readiness: head-pair hp only needs the dh=hp
                # halves of its qT/kT blocks.
                done = 0
                while st_pre["i"] < len(pre_order) and done < limit:
                    sqb, hp, sk, eng = pre_order[st_pre["i"]]
                    if (sqb, hp) not in st_pre["q"] or \
                            (sk // 4, hp) not in st_pre["k"]:
                        break
                    pre[(sqb, hp, sk)] = qk_exp_pre(hp, sqb, sk, eng)
                    st_pre["i"] += 1
                    done += 1

            def proj_qk(xd, w, bias2, dst, sb, which):
                xt = load_xt(xd, sb, "xt" + which)
                for dh in range(2):
                    proj_dmajor_unit(xt, w, bias2, dst, sb, dh)
                    st_pre[which].add((sb, dh))
                    emit_pre(3)

            # batch (0,0)'s AV interleaves into the V phase: its et tiles
            # are all precomputed, and vS[sk] is ready right after block
            # sk//4's epilogue -- so the first attention batch costs no
            # wall-clock of its own.
            pav00 = [ps_av.tile([128, 4, 128], F32, tag="av", name=f"pav{hh}")
                     for hh in range(2)]

            def proj_v(sb):
                # batch (0,0)'s AV trails the V epilogues by one si unit so
                # the in-order PE never waits on the DVE vS write latency.
                xtv = load_xt(xv, sb, "xtv")
                for si in range(4):
                    pv = ps_pj.tile([128, 512], F32, tag="pj", name="pv")
                    for j in range(HT):
                        nc.tensor.matmul(pv[:, 0:D],
                                         xtv[:, j, ts(si, 128)],
                                         wv[:, j, :],
                                         start=(j == 0), stop=(j == HT - 1))
                    nc.vector.tensor_add(
                        vS[:, 4 * sb + si, :, 0:HD],
                        pv[:, 0:D].rearrange("p (g d) -> p g d", g=4),
                        bvb[:].rearrange("p (g d) -> p g d", g=4))
                    emit_pre(3)
                    sk = 4 * sb + si
                    if sk > 0:
                        av_accum(0, sk - 1, pre.pop((0, 0, sk - 1)), pav00)

            # Q0 then all K (unlocks every sqb0 tile), then Q1-3 (unlocks
            # the rest), V last (first consumed only once attention starts).
            # xtq0 lands in two halves so the first projection matmul can
            # start on the first 256 columns while the rest streams in.
            xtq0 = xt_p.tile([128, HT, SB], BF16, tag="xt", name="xtq0")
            xq_r = xq.rearrange("(j p) s -> p j s", p=128)
            nc.sync.dma_start(xtq0[:, :, 0:256], xq_r[:, :, 0:256])
            nc.sync.dma_start(xtq0[:, :, 256:512], xq_r[:, :, 256:512])
            nc.scalar.dma_start(bq2[:], bq_d[:])
            wk_r = wk_d.rearrange("(j p) d -> p j d", p=128)
            nc.scalar.dma_start(wk[:, :, 0:128], wk_r[:, :, 0:128])
            # fast start: narrow first xk load + mini K projection puts the
            # first score+exp on the engines early.
            xtk0a = xt_p.tile([128, HT, 128], BF16, tag="xta", name="xtk0a", bufs=1)
            nc.sync.dma_start(wq[:, :, 128:256], wq_r[:, :, 128:256])
            nc.sync.dma_start(
                xtk0a[:], xk.rearrange("(j p) s -> p j s", p=128)[:, :, 0:128])
            nc.scalar.dma_start(bk2[:], bk_d[:])
            nc.scalar.dma_start(wk[:, :, 128:256], wk_r[:, :, 128:256])
            proj_dmajor_unit(xtq0, wq, bq2, qT, 0, 0, 0, 256)
            proj_dmajor_unit(xtq0, wq, bq2, qT, 0, 0, 256, 512)
            proj_dmajor_unit(xtq0, wq, bq2, qT, 0, 1)
            st_pre["q"].add((0, 0))
            st_pre["q"].add((0, 1))
            pk0 = ps_pj.tile([128, 512], F32, tag="pj", name="pk0")
            for j in range(HT):
                nc.tensor.matmul(pk0[:, 0:128], wk[:, j, 0:128],
                                 xtk0a[:, j, :],
                                 start=(j == 0), stop=(j == HT - 1))
            nc.vector.tensor_scalar_add(kT[:, 0, 0:128], pk0[:, 0:128],
                                        bk2[:, 0:1])
            pre[(0, 0, 0)] = qk_exp_pre(0, 0, 0, "aa")
            st_pre["i"] = 1
            xtk0 = load_xt(xk, 0, "xtk")
            proj_dmajor_unit(xtk0, wk, bk2, kT, 0, 0, 128, SB)
            st_pre["k"].add((0, 0))
            emit_pre(3)
            proj_dmajor_unit(xtk0, wk, bk2, kT, 0, 1)
            st_pre["k"].add((0, 1))
            emit_pre(3)
            for sb in range(1, NSB):
                proj_qk(xk, wk, bk2, kT, sb, "k")
                emit_pre(3)
            for sb in range(1, NSB):
                proj_qk(xq, wq, bq2, qT, sb, "q")
                emit_pre(3)
            nc.sync.dma_start(wv[:], wv_d.rearrange("(j p) d -> p j d", p=128))
            # v-bias broadcast, deferred here so its small DMAs stay off the
            # critical startup path (first needed by V0's epilogue)
            nc.sync.dma_start(bv1[:], bv_d[:])
            nc.vector.tensor_copy(bv1r[:], bv1[:])
            pbc = ps_pj.tile([128, 512], F32, tag="pj", name="pbc")
            nc.tensor.matmul(pbc[:, 0:D], ones[0:1, :], bv1r[:])
            nc.vector.tensor_copy(bvb[:], pbc[:, 0:D])
            for sb in range(NSB):
                proj_v(sb)
            av_accum(0, SKT - 1, pre.pop((0, 0, SKT - 1)), pav00)
            emit_pre(len(pre_order))

            # deferred: output-projection weights (first needed ~60us in)
            wo = const.tile([128, 2, H], BF16)
            nc.sync.dma_start(wo[:], wo_d.rearrange("(i p) e -> p i e", p=128))

            # ---- attention stream. Inline tiles split each head pair's
            # exp across ACT/DVE (separate PSUM banks), with 2-item score
            # lookahead across batch boundaries. outproj(sqb-1) chunks are
            # spread through the following batch (sk = 2,5,8,11) so the
            # ps_pj ring never backs up on the ACT drain. ----
            attn_finish(0, 0, pav00)
            batches = [(s, h) for s in range(NSB) for h in range(2)][1:]
            stream = [(s, h, k) for (s, h) in batches for k in range(SKT)]
            emitted = {}
            st_la = {"ep": 0}

            def pump(upto, inline_ahead=0):
                # emit score+exp for stream items <= upto, plus keep
                # `inline_ahead` INLINE (non-pre) tiles in flight beyond
                # the consumer. `emitted` holds exactly the un-consumed
                # inline tiles, so len(emitted) IS the in-flight count --
                # pre-covered items don't eat the lookahead window.
                while st_la["ep"] < len(stream) and \
                        (st_la["ep"] <= upto or len(emitted) < inline_ahead):
                    key = stream[st_la["ep"]]
                    if key not in pre:
                        emitted[key] = qk_exp_split(key[1], key[0], key[2])
                    st_la["ep"] += 1

            deferred = []
            pending_drain = []
            pav_cur = {}
            for ci, key in enumerate(stream):
                sqb, hp, sk = key
                if sk == 0:
                    pav_cur[(sqb, hp)] = [
                        ps_av.tile([128, 4, 128], F32, tag="av",
                                   name=f"pav{hh}") for hh in range(2)]
                # ensure the current tile's exp exists; the lookahead pump
                # at loop end runs AFTER any finish so normalize ops aren't
                # queued behind the next batch's exps.
                pump(ci)
                if pending_drain:
                    emit_outproj_drain(pending_drain.pop(0))
                et = pre.pop(key, None)
                if et is None:
                    et = emitted.pop(key)
                av_accum(hp, sk, et, pav_cur[(sqb, hp)])
                if deferred and sk in (2, 5, 8, 11):
                    dq, dst = deferred[0]
                    pending_drain.append(emit_outproj_mm(dq, dst))
                    if dst == 3:
                        deferred.pop(0)
                    else:
                        deferred[0] = (dq, dst + 1)
                if sk == SKT - 1:
                    last = (sqb == NSB - 1 and hp == 1)
                    attn_finish(hp, sqb, pav_cur.pop((sqb, hp)),
                                fuse_outproj=last)
                    if hp == 0 and sqb > 0:
                        deferred.append((sqb - 1, 0))
                pump(ci, inline_ahead=2)

    nc.compile()
    return nc


def _get_nc():
    if "nc" not in _CACHE:
        _CACHE["nc"] = _build()
    return _CACHE["nc"]


def _kernel_numpy(query, key, value, attention_mask,
                  Wq, bq, Wk, bk, Wv, bv, Wo, bo):
    """Exact fp32 numpy fallback (only used for inputs outside the spec:
    nonzero mask or unexpected shapes)."""
    B, S_, H_ = query.shape
    NH = 16
    HDl = H_ // NH
    q = query @ Wq + bq
    k = key @ Wk + bk
    v = value @ Wv + bv

    def split(x):
        return x.reshape(B, S_, NH, HDl).transpose(0, 2, 1, 3)

    q, k, v = split(q), split(k), split(v)
    s = np.einsum("bhqd,bhkd->bhqk", q, k) / np.sqrt(np.float32(HDl))
    s = s + attention_mask[:, None, :, :]
    s = s - s.max(axis=-1, keepdims=True)
    e = np.exp(s)
    w = e / e.sum(axis=-1, keepdims=True)
    o = np.einsum("bhqk,bhkd->bhqd", w, v)
    o = o.transpose(0, 2, 1, 3).reshape(B, S_, H_)
    return (o @ Wo + bo).astype(np.float32)


def kernel(query, key, value, attention_mask, Wq, bq, Wk, bk, Wv, bv, Wo, bo):
    query = np.asarray(query, np.float32)
    key = np.asarray(key, np.float32)
    value = np.asarray(value, np.float32)
    Wq, Wk, Wv, Wo = (np.asarray(a, np.float32) for a in (Wq, Wk, Wv, Wo))
    bq, bk, bv, bo = (np.asarray(a, np.float32) for a in (bq, bk, bv, bo))
    attention_mask = np.asarray(attention_mask, np.float32)

    if query.shape != (2, S, H) or Wq.shape != (H, H) or \
            attention_mask.shape != (2, S, S) or np.any(attention_mask):
        return _kernel_numpy(query, key, value, attention_mask,
                             Wq, bq, Wk, bk, Wv, bv, Wo, bo)

    qT = [np.ascontiguousarray(query[b].astype(ml_dtypes.bfloat16).T)
          for b in range(2)]
    kTh = [np.ascontiguousarray(key[b].astype(ml_dtypes.bfloat16).T)
           for b in range(2)]
    vTh = [np.ascontiguousarray(value[b].astype(ml_dtypes.bfloat16).T)
           for b in range(2)]

    nc = _get_nc()
    in_maps = []
    for c in range(N_CORES):
        b, hg = divmod(c, 4)
        sl = slice(D * hg, D * hg + D)
        in_maps.append({
            "xqT": qT[b],
            "xkT": kTh[b],
            "xvT": vTh[b],
            "wq": np.ascontiguousarray(Wq[:, sl]).astype(ml_dtypes.bfloat16),
            "wk": np.ascontiguousarray(Wk[:, sl]).astype(ml_dtypes.bfloat16),
            "wv": np.ascontiguousarray(Wv[:, sl]).astype(ml_dtypes.bfloat16),
            "wo": np.ascontiguousarray(Wo[sl, :]).astype(ml_dtypes.bfloat16),
            "bq2": bq[sl].reshape(2, 128).T.copy(),
            "bk2": bk[sl].reshape(2, 128).T.copy(),
            "bv1": bv[sl].reshape(1, D).copy(),
        })
    try:
        res = run_bass_kernel_spmd(nc, in_maps, list(range(N_CORES)))
    finally:
        # run_bass_via_pjrt monkeypatches libneuronxla.neuronx_cc; restore it
        # so later ordinary jax compiles in the caller's process are untouched.
        try:
            import libneuronxla  # pyright: ignore[reportMissingImports]
            if hasattr(libneuronxla, "orig_neuronx_cc"):
                libneuronxla.neuronx_cc = libneuronxla.orig_neuronx_cc
        except ImportError:
            pass
    outs = [res.results[c]["y"] for c in range(N_CORES)]
    out = np.empty((2, S, H), np.float32)
    for b in range(2):
        out[b] = (outs[4 * b].astype(np.float32)
                  + outs[4 * b + 1].astype(np.float32)
                  + outs[4 * b + 2].astype(np.float32)
                  + outs[4 * b + 3].astype(np.float32)) + bo
    return out
